# revision 14
# baseline (speedup 1.0000x reference)
"""Trainium2 Bass kernel for CausalSelfAttention with learned TxT score mixing.

Reference computation (per batch b):
    v = x @ Wv.T + bv ; q = k = v
    S = (v_h @ v_h.T) / sqrt(hd)            per head h   [T,T]
    A = S @ Wl.T @ Wc.T + bc                             [T,T]
    P = softmax(causal_mask(A))                          [T,T]
    y_h = P @ v_h ; out = concat(y) @ Wp.T + bp

Key algebra:
  * q == k == v makes S rank-64, so the TxT mixing collapses:
    A.T = Wc @ Wl @ (v_h v_h.T)/8 = ((Wc/8) @ (Wl @ v_h)) @ v_h.T = U_h @ v_h.T.
    We compute G_h = Wl @ v_h (per head, pair-packed) and U_h = (Wc/8) @ G_h
    directly -- no TxT @ TxT precompute at all.  U.T comes out of the second
    matmul in exactly the [d, j] layout the score matmuls need.
  * Scores stay in [key, query] layout; softmax uses unnormalized exp() and
    the normalizer Z[q] falls out of the PV matmul via a ones-column in the
    stationary operand (logits are O(1); masked entries are exactly zero).
  * Causal structure at 128 granularity: exp() runs only on the valid
    q-range of each key tile, the 0/1 mask multiply touches only the 128-wide
    diagonal band, and the fully-masked region is memset to zero.
  * Per-pair normalization: 1/Z rows are partition-broadcast with a tiny
    K=2 matmul (no DRAM round-trips), so the tail of the kernel is short.
  * Matmul operands are bf16 (accumulation, softmax and bias math in fp32);
    head pairs are row/col-packed into the 128-wide PE array; per-pair PV
    chains are software-pipelined against the next pair's score/exp stream.

Sharding: data-parallel over batch (core b <- batch b). All weights
replicated; host passes transposed bf16 copies (layout/dtype prep only).
"""

import os
import sys

for _p in ("/opt/trn_rl_repo", "/root/.axon_site/_ro/trn_rl_repo"):
    if os.path.isdir(_p) and _p not in sys.path:
        sys.path.insert(0, _p)

import numpy as np

import concourse.bass as bass
import concourse.tile as tile
from concourse import bacc, mybir
from concourse.bass_utils import run_bass_kernel_spmd

B, T, C, NH = 8, 1024, 768, 12
HD = C // NH          # 64
P = 128               # SBUF partitions
NJT = T // P          # 8 j/k tiles of 128
NCT = C // P          # 6 c tiles of 128
NPAIR = NH // 2       # 6 head pairs (two heads share a 128-partition tile)
QG = 512              # q granularity for scores/exp/PV (= q slice width)
NQS = T // QG         # 2
FDT = QG
NMSK = QG // P        # 4 diagonal-tile positions per q slice
HC = HD + 1           # 65: head value columns + ones column for Z
DT = mybir.dt.float32
BF = mybir.dt.bfloat16
MM_DT = BF            # dtype of all matmul operands (accumulation stays fp32)

LAST_EXEC_NS = None
LAST_RESULTS = None


def _emit(nc):
    """Emit the whole per-core program inside a TileContext."""
    xT = nc.dram_tensor("xT", [C, T], MM_DT, kind="ExternalInput")
    WvT = nc.dram_tensor("WvT", [C, C], MM_DT, kind="ExternalInput")
    WlT = nc.dram_tensor("WlT", [T, T], MM_DT, kind="ExternalInput")
    WcT = nc.dram_tensor("WcT", [T, T], MM_DT, kind="ExternalInput")  # pre-scaled /8
    WpT = nc.dram_tensor("WpT", [C, C], MM_DT, kind="ExternalInput")
    bv = nc.dram_tensor("bv", [C], DT, kind="ExternalInput")
    bc = nc.dram_tensor("bc", [T], DT, kind="ExternalInput")
    bp = nc.dram_tensor("bp", [C], DT, kind="ExternalInput")
    mask128 = nc.dram_tensor("mask128", [P, P], MM_DT, kind="ExternalInput")
    outT = nc.dram_tensor("outT", [C, T], DT, kind="ExternalOutput")

    with tile.TileContext(nc) as tc:
        with tc.tile_pool(name="consts", bufs=1) as consts:
            vT_sb = consts.tile([P, NCT, T], MM_DT)          # v.T  [c, t]
            v_sb = consts.tile([P, NJT, NH * HC], MM_DT)     # v    [t, h*65+d], col 64 = ones
            uT_sb = consts.tile([P, NPAIR, T], MM_DT)        # U.T pair-packed [hh*64+d, hp, j]
            yT_sb = consts.tile([P, NCT, T], MM_DT)          # normalized attn out, [c, t]
            wpT_t = consts.tile([P, NCT, C], MM_DT)          # Wp.T[c, c_out]
            mask_sb = consts.tile([P, P], MM_DT)             # lower-tri [jl, qq]: jl<=qq
            ones1_sb = consts.tile([1, HD], MM_DT)           # K=1 broadcast stationary
            bc_sb = consts.tile([P, NJT], DT)
            bv_sb = consts.tile([P, NCT], DT)
            bvbc_sb = consts.tile([P, C], DT)                # bv broadcast across partitions
            bp_sb = consts.tile([P, NCT], DT)

            # ones columns of v_sb (column h*65+64 <- 1.0), for the Z row of PV
            for tt in range(NJT):
                nc.vector.memset(
                    v_sb[:, tt].rearrange("p (h e) -> p h e", e=HC)[:, :, HD : HD + 1],
                    1.0,
                )
            nc.vector.memset(ones1_sb, 1.0)

            # ---------------- phase 1: v (natural) and v.T projections ------------
            with (
                tc.tile_pool(name="ph1", bufs=1) as ph1,
                tc.tile_pool(name="ps1", bufs=8, space="PSUM") as ps1,
            ):
                xT_t = ph1.tile([P, NCT, T], MM_DT)
                wvT_t = ph1.tile([P, NCT, C], MM_DT)
                # DMA priority order: wvT/xT first (phase-1 critical path)
                for ck in range(NCT):
                    nc.sync.dma_start(out=wvT_t[:, ck], in_=WvT[ck * P : (ck + 1) * P, :])
                    nc.sync.dma_start(out=xT_t[:, ck], in_=xT[ck * P : (ck + 1) * P, :])
                nc.sync.dma_start(out=bc_sb, in_=bc[:].rearrange("(jt p) -> p jt", p=P))
                nc.sync.dma_start(out=bv_sb, in_=bv[:].rearrange("(ct p) -> p ct", p=P))
                nc.sync.dma_start(out=bp_sb, in_=bp[:].rearrange("(ct p) -> p ct", p=P))
                nc.sync.dma_start(out=mask_sb, in_=mask128[:, :])
                bv_ap = bv[:]
                nc.gpsimd.dma_start(
                    out=bvbc_sb,
                    in_=bass.AP(
                        tensor=bv_ap.tensor, offset=bv_ap.offset, ap=[[0, P]] + list(bv_ap.ap)
                    ),
                )

                # v.T[c, t] = sum_c' Wv[c, c'] x[t, c']  (+ bv[c] per-partition)
                for ts in range(2):
                    pts = [ps1.tile([P, FDT], DT, tag="pts1", name="pts1") for _ in range(NCT)]
                    for ck in range(NCT):
                        for ct in range(NCT):
                            nc.tensor.matmul(
                                pts[ct],
                                wvT_t[:, ck, ct * P : (ct + 1) * P],
                                xT_t[:, ck, ts * FDT : (ts + 1) * FDT],
                                start=(ck == 0),
                                stop=(ck == NCT - 1),
                            )
                    for ct in range(NCT):
                        nc.vector.tensor_scalar_add(
                            vT_sb[:, ct, ts * FDT : (ts + 1) * FDT],
                            pts[ct],
                            bv_sb[:, ct : ct + 1],
                        )

                # v[t, c] = sum_c' x[t, c'] Wv[c, c']  (+ bv[c] broadcast)
                cslices = [(0, FDT), (FDT, C - FDT)]
                for half in range(2):
                    pts2 = [ps1.tile([P, FDT], DT, tag="pts1", name="pts1") for _ in range(8)]
                    for ck in range(NCT):
                        for i in range(4):
                            tt = half * 4 + i
                            for si, (c0, cw) in enumerate(cslices):
                                nc.tensor.matmul(
                                    pts2[i * 2 + si][:, :cw],
                                    xT_t[:, ck, tt * P : (tt + 1) * P],
                                    wvT_t[:, ck, c0 : c0 + cw],
                                    start=(ck == 0),
                                    stop=(ck == NCT - 1),
                                )
                    for i in range(4):
                        tt = half * 4 + i
                        vdst = v_sb[:, tt].rearrange("p (h e) -> p h e", e=HC)
                        for si, (c0, cw) in enumerate(cslices):
                            nh0, nh1 = c0 // HD, (c0 + cw) // HD
                            nc.vector.tensor_add(
                                vdst[:, nh0:nh1, 0:HD],
                                pts2[i * 2 + si][:, :cw].rearrange(
                                    "p (h e) -> p h e", e=HD
                                ),
                                bvbc_sb[:, c0 : c0 + cw].rearrange(
                                    "p (h e) -> p h e", e=HD
                                ),
                            )

            # ---------------- phase 2a: G = Wl @ v ; U.T = (G.T @ (Wc/8).T).T -------
            # G[m, d] = sum_t Wl[m, t] v[t, d]       lhsT = Wl.T[t, m]  rhs = v[t, d]
            # U.T[d, j] = sum_m G[m, d] WcT[m, j]    lhsT = G[m, d]     rhs = WcT[m, j]
            # (the /8 score scale is folded into WcT on the host)
            with (
                tc.tile_pool(name="ph2a", bufs=1) as ph2a,
                tc.tile_pool(name="ps2", bufs=2, space="PSUM") as ps2,
            ):
                wlT_t = ph2a.tile([P, NJT, T], MM_DT)   # Wl.T[t, m]
                wcT_t = ph2a.tile([P, NJT, T], MM_DT)   # Wc.T[m, j] (pre-scaled)
                g_sb = ph2a.tile([P, NJT, C], MM_DT)    # G[m, d] m-tile major
                for mt in range(NJT):
                    nc.sync.dma_start(out=wlT_t[:, mt], in_=WlT[mt * P : (mt + 1) * P, :])
                    nc.sync.dma_start(out=wcT_t[:, mt], in_=WcT[mt * P : (mt + 1) * P, :])
                for ck in range(NCT):
                    nc.sync.dma_start(out=wpT_t[:, ck], in_=WpT[ck * P : (ck + 1) * P, :])

                # G: for each m-tile, accumulate over t; rhs walks v's head columns
                # (strided AP skipping the ones columns), split 512 + 256.
                hslices = [(0, 8), (8, 4)]
                for mt in range(NJT):
                    gp = [
                        ps2.tile([P, nh * HD], DT, tag="gp", name="gp")
                        for _, nh in hslices
                    ]
                    for kt in range(NJT):
                        vv = v_sb[:, kt].rearrange("p (h e) -> p h e", e=HC)
                        for si, (h0, nh) in enumerate(hslices):
                            nc.tensor.matmul(
                                gp[si],
                                wlT_t[:, kt, mt * P : (mt + 1) * P],
                                vv[:, h0 : h0 + nh, 0:HD],
                                start=(kt == 0),
                                stop=(kt == NJT - 1),
                            )
                    for si, (h0, nh) in enumerate(hslices):
                        nc.vector.tensor_copy(
                            g_sb[:, mt, h0 * HD : (h0 + nh) * HD], gp[si]
                        )

                # U.T pair-packed: lhsT = G[m, pair cols], rhs = WcT[m, j slice]
                for hp in range(NPAIR):
                    for js in range(NQS):
                        up = ps2.tile([P, FDT], DT, tag="up", name="up")
                        for mt in range(NJT):
                            nc.tensor.matmul(
                                up,
                                g_sb[:, mt, hp * P : (hp + 1) * P],
                                wcT_t[:, mt, js * FDT : (js + 1) * FDT],
                                start=(mt == 0),
                                stop=(mt == NJT - 1),
                            )
                        nc.vector.tensor_copy(
                            uT_sb[:, hp, js * FDT : (js + 1) * FDT], up
                        )

            # ---------------- phase 2b: scores -> exp -> mask -> PV -> norm --------
            with (
                tc.tile_pool(name="sm", bufs=8) as sm,
                tc.tile_pool(name="p_pool", bufs=3) as p_pool,
                tc.tile_pool(name="outp", bufs=4) as outp,
                tc.tile_pool(name="a_ps", bufs=2, space="PSUM") as a_ps,
                tc.tile_pool(name="y_ps", bufs=2, space="PSUM") as y_ps,
                tc.tile_pool(name="ps3", bufs=2, space="PSUM") as ps3,
            ):
                def emit_proj(ts, cts=range(NCT)):
                    # outT[c_out, t] = Wp @ yT (+bp), ct-outer accumulation chains
                    for ct in cts:
                        pp = ps3.tile([P, FDT], DT, tag="pp")
                        for ck in range(NCT):
                            nc.tensor.matmul(
                                pp,
                                wpT_t[:, ck, ct * P : (ct + 1) * P],
                                yT_sb[:, ck, ts * FDT : (ts + 1) * FDT],
                                start=(ck == 0),
                                stop=(ck == NCT - 1),
                            )
                        ot = outp.tile([P, FDT], DT, tag="ot")
                        nc.vector.tensor_scalar_add(ot, pp, bp_sb[:, ct : ct + 1])
                        nc.sync.dma_start(
                            out=outT[ct * P : (ct + 1) * P, ts * FDT : (ts + 1) * FDT],
                            in_=ot,
                        )

                def pv_gen(hp, qs, pb, jmax):
                    """Generator emitting the PV chains + per-pair normalization;
                    driven interleaved with the NEXT pair's score stream so the
                    PE keeps busy while ACT runs this pair's exps."""
                    q0 = qs * FDT
                    zrec = [
                        sm.tile([1, FDT], MM_DT, tag=f"zrec{hh}", name="zrec")
                        for hh in range(2)
                    ]
                    yu = sm.tile([P, FDT], MM_DT, tag="yu", name="yu", bufs=2)
                    for hh in range(2):
                        h = hp * 2 + hh
                        yp = y_ps.tile([HC, QG], DT, tag="yp", name="yp")
                        for kt in range(jmax + 1):
                            nc.tensor.matmul(
                                yp,
                                v_sb[:, kt, h * HC : (h + 1) * HC],
                                pb[:, hh, kt],
                                start=(kt == 0),
                                stop=(kt == jmax),
                            )
                            yield
                        # 1/Z for this head (bf16 row, feeds the broadcast mm)
                        with nc.allow_low_precision(reason="1/Z broadcast operand"):
                            nc.vector.reciprocal(zrec[hh], yp[HD : HD + 1, :])
                        if hh == 0:
                            nc.vector.tensor_copy(yu[0:HD, :], yp[0:HD, :])
                        else:
                            stg = sm.tile([HD, QG], MM_DT, tag="stg", name="stg")
                            nc.vector.tensor_copy(stg, yp[0:HD, :])
                            nc.sync.dma_start(out=yu[HD:P, :], in_=stg)
                        yield
                    # partition-broadcast of 1/Z via two K=1 matmuls, then normalize
                    rbb = ps3.tile([P, FDT], DT, tag="pp", name="rbb")
                    nc.tensor.matmul(
                        rbb[0:HD, :], ones1_sb, zrec[0], start=True, stop=True
                    )
                    nc.tensor.matmul(
                        rbb[HD:P, :],
                        ones1_sb,
                        zrec[1],
                        start=True,
                        stop=True,
                        tile_position=(0, HD),
                    )
                    nc.vector.tensor_mul(yT_sb[:, hp, q0 : q0 + FDT], rbb, yu)
                    yield

                def exhaust(g):
                    if g is not None:
                        for _ in g:
                            pass

                prev_gen = None
                jobs = [(1, hp) for hp in range(NPAIR)] + [(0, hp) for hp in range(NPAIR)]
                for qs, hp in jobs:
                    q0 = qs * FDT
                    jmax = NMSK * qs + NMSK - 1
                    if qs == 0 and hp in (1, 2, 3, 4, 5):
                        i = hp - 1
                        hi = NCT if hp == 5 else i + 1
                        emit_proj(1, range(i, hi))  # big slice's projection as filler
                    # scores: A.T[j, q] single K=64 matmuls, row-packed pairs;
                    # exp on the valid q-range only; mask-mult on the diag band
                    pb = p_pool.tile(
                        [P, 2, NJT, FDT], MM_DT, tag="pb", name="pb"
                    )
                    for jt in range(jmax + 1):
                        jrel = jt - NMSK * qs  # diag position (>=0 on diag tiles)
                        bs = max(jrel, 0) * P  # first valid q column
                        if bs > 0:
                            nc.vector.memset(pb[:, :, jt, 0:bs], 0.0)
                        ap2 = a_ps.tile([P, 2, FDT], DT, tag="ap2", name="ap2")
                        for hh in range(2):
                            lo = hh * HD
                            nc.tensor.matmul(
                                ap2[:, hh, :],
                                uT_sb[lo : lo + HD, hp, jt * P : (jt + 1) * P],
                                vT_sb[lo : lo + HD, hp, q0 : q0 + FDT],
                                start=True,
                                stop=True,
                            )
                        nc.scalar.activation(
                            pb[:, :, jt, bs:FDT],
                            ap2[:, :, bs:FDT],
                            mybir.ActivationFunctionType.Exp,
                            bias=bc_sb[:, jt : jt + 1],
                        )
                        if jrel >= 0:  # diagonal tile: mask the 128-wide band
                            for hh in range(2):
                                nc.vector.tensor_mul(
                                    pb[:, hh, jt, bs : bs + P],
                                    pb[:, hh, jt, bs : bs + P],
                                    mask_sb,
                                )
                        if prev_gen is not None:
                            for _ in range(4 if qs == 1 else 6):
                                if next(prev_gen, "end") == "end":
                                    prev_gen = None
                                    break
                    exhaust(prev_gen)
                    prev_gen = pv_gen(hp, qs, pb, jmax)
                exhaust(prev_gen)
                emit_proj(0)

    return nc

_NC = None


def build_nc():
    global _NC
    if _NC is None:
        nc = bacc.Bacc("TRN2", target_bir_lowering=False, debug=False)
        _emit(nc)
        nc.compile()
        _NC = nc
    return _NC


def make_mask128():
    import ml_dtypes

    m = (np.arange(P)[:, None] <= np.arange(P)[None, :]).astype(np.float32)
    return m.astype(ml_dtypes.bfloat16)


def host_inputs(x, Wv, bv, Wl, Wc, bc, Wp, bp):
    """Per-core input maps: layout/dtype prep (transposes + bf16 casts)."""
    import ml_dtypes

    bf16 = ml_dtypes.bfloat16
    x = np.ascontiguousarray(np.asarray(x, dtype=np.float32))
    shared = {
        "WvT": np.ascontiguousarray(np.asarray(Wv, np.float32).T.astype(bf16)),
        "WlT": np.ascontiguousarray(np.asarray(Wl, np.float32).T.astype(bf16)),
        "WcT": np.ascontiguousarray(
            (np.asarray(Wc, np.float32).T / np.sqrt(np.float32(HD))).astype(bf16)
        ),
        "WpT": np.ascontiguousarray(np.asarray(Wp, np.float32).T.astype(bf16)),
        "bv": np.ascontiguousarray(np.asarray(bv, np.float32)),
        "bc": np.ascontiguousarray(np.asarray(bc, np.float32)),
        "bp": np.ascontiguousarray(np.asarray(bp, np.float32)),
        "mask128": make_mask128(),
    }
    return [
        {"xT": np.ascontiguousarray(x[b].T.astype(bf16)), **shared} for b in range(B)
    ]


def kernel(x, Wv, bv, Wl, Wc, bc, Wp, bp, _trace=False):
    global LAST_EXEC_NS, LAST_RESULTS
    nc = build_nc()
    in_maps = host_inputs(x, Wv, bv, Wl, Wc, bc, Wp, bp)
    rb = run_bass_kernel_spmd(nc, in_maps, list(range(B)), trace=_trace)
    LAST_EXEC_NS = rb.exec_time_ns
    LAST_RESULTS = rb
    out = np.stack([rb.results[b]["outT"].T for b in range(B)], axis=0)
    return np.ascontiguousarray(out.astype(np.float32))


# revision 19
# speedup vs baseline: 1.2878x; 1.2878x over previous
"""Trainium2 Bass kernel for CausalSelfAttention with learned TxT score mixing.

Reference computation (per batch b):
    v = x @ Wv.T + bv ; q = k = v
    S = (v_h @ v_h.T) / sqrt(hd)            per head h   [T,T]
    A = S @ Wl.T @ Wc.T + bc                             [T,T]
    P = softmax(causal_mask(A))                          [T,T]
    y_h = P @ v_h ; out = concat(y) @ Wp.T + bp

Key algebra:
  * q == k == v makes S rank-64, so the TxT mixing collapses:
    A.T = Wc @ Wl @ (v_h v_h.T)/8 = ((Wc/8) @ (Wl @ v_h)) @ v_h.T = U_h @ v_h.T.
    We compute G_h = Wl @ v_h (per head, pair-packed) and U_h = (Wc/8) @ G_h
    directly -- no TxT @ TxT precompute at all.  U.T comes out of the second
    matmul in exactly the [d, j] layout the score matmuls need.
  * Scores stay in [key, query] layout; softmax uses unnormalized exp() and
    the normalizer Z[q] falls out of the PV matmul via a ones-column in the
    stationary operand (logits are O(1); masked entries are exactly zero).
  * Causal structure at 128 granularity: exp() runs only on the valid
    q-range of each key tile, the 0/1 mask multiply touches only the 128-wide
    diagonal band, and the fully-masked region is memset to zero.
  * Per-pair normalization: 1/Z rows are partition-broadcast with a tiny
    K=2 matmul (no DRAM round-trips), so the tail of the kernel is short.
  * Matmul operands are bf16 (accumulation, softmax and bias math in fp32);
    head pairs are row/col-packed into the 128-wide PE array; per-pair PV
    chains are software-pipelined against the next pair's score/exp stream.

Sharding: data-parallel over batch (core b <- batch b). All weights
replicated; host passes transposed bf16 copies (layout/dtype prep only).
"""

import os
import sys

for _p in ("/opt/trn_rl_repo", "/root/.axon_site/_ro/trn_rl_repo"):
    if os.path.isdir(_p) and _p not in sys.path:
        sys.path.insert(0, _p)

import numpy as np

import concourse.bass as bass
import concourse.tile as tile
from concourse import bacc, mybir
from concourse.bass_utils import run_bass_kernel_spmd

B, T, C, NH = 8, 1024, 768, 12
HD = C // NH          # 64
P = 128               # SBUF partitions
NJT = T // P          # 8 j/k tiles of 128
NCT = C // P          # 6 c tiles of 128
NPAIR = NH // 2       # 6 head pairs (two heads share a 128-partition tile)
QG = 512              # q granularity for scores/exp/PV (= q slice width)
NQS = T // QG         # 2
FDT = QG
NMSK = QG // P        # 4 diagonal-tile positions per q slice
HC = HD + 1           # 65: head value columns + ones column for Z
DT = mybir.dt.float32
BF = mybir.dt.bfloat16
MM_DT = BF            # dtype of all matmul operands (accumulation stays fp32)

LAST_EXEC_NS = None
LAST_RESULTS = None


def _emit(nc):
    """Emit the whole per-core program inside a TileContext."""
    xT = nc.dram_tensor("xT", [C, T], MM_DT, kind="ExternalInput")
    WvT = nc.dram_tensor("WvT", [C, C], MM_DT, kind="ExternalInput")
    WlT = nc.dram_tensor("WlT", [T, T], MM_DT, kind="ExternalInput")
    WcT = nc.dram_tensor("WcT", [T, T], MM_DT, kind="ExternalInput")  # pre-scaled /8
    WpT = nc.dram_tensor("WpT", [C, C], MM_DT, kind="ExternalInput")
    bv = nc.dram_tensor("bv", [C], DT, kind="ExternalInput")
    bc = nc.dram_tensor("bc", [T], DT, kind="ExternalInput")
    bp = nc.dram_tensor("bp", [C], DT, kind="ExternalInput")
    mask128 = nc.dram_tensor("mask128", [P, P], MM_DT, kind="ExternalInput")
    outT = nc.dram_tensor("outT", [C, T], DT, kind="ExternalOutput")

    with tile.TileContext(nc) as tc:
        with tc.tile_pool(name="consts", bufs=1) as consts:
            vT_sb = consts.tile([P, NCT, T], MM_DT)          # v.T  [c, t]
            v_sb = consts.tile([P, NJT, NH * HC], MM_DT)     # v    [t, h*65+d], col 64 = ones
            uT_sb = consts.tile([P, NPAIR, T], MM_DT)        # U.T pair-packed [hh*64+d, hp, j]
            yT_sb = consts.tile([P, NCT, T], MM_DT)          # normalized attn out, [c, t]
            wpT_t = consts.tile([P, NCT, C], MM_DT)          # Wp.T[c, c_out]
            mask_sb = consts.tile([P, P], MM_DT)             # lower-tri [jl, qq]: jl<=qq
            ones1_sb = consts.tile([1, HD], MM_DT)           # K=1 broadcast stationary
            bc_sb = consts.tile([P, NJT], DT)
            bv_sb = consts.tile([P, NCT], DT)
            bvbc_sb = consts.tile([P, C], DT)                # bv broadcast across partitions
            bp_sb = consts.tile([P, NCT], DT)

            # ones columns of v_sb (column h*65+64 <- 1.0), for the Z row of PV
            for tt in range(NJT):
                nc.vector.memset(
                    v_sb[:, tt].rearrange("p (h e) -> p h e", e=HC)[:, :, HD : HD + 1],
                    1.0,
                )
            nc.vector.memset(ones1_sb, 1.0)

            # ---------------- phase 1: v (natural) and v.T projections ------------
            with (
                tc.tile_pool(name="ph1", bufs=1) as ph1,
                tc.tile_pool(name="ps1", bufs=8, space="PSUM") as ps1,
            ):
                xT_t = ph1.tile([P, NCT, T], MM_DT)
                wvT_t = ph1.tile([P, NCT, C], MM_DT)
                # DMA priority order: wvT/xT first (phase-1 critical path)
                for ck in range(NCT):
                    nc.sync.dma_start(out=wvT_t[:, ck], in_=WvT[ck * P : (ck + 1) * P, :])
                    nc.sync.dma_start(out=xT_t[:, ck], in_=xT[ck * P : (ck + 1) * P, :])
                nc.sync.dma_start(out=bc_sb, in_=bc[:].rearrange("(jt p) -> p jt", p=P))
                nc.sync.dma_start(out=bv_sb, in_=bv[:].rearrange("(ct p) -> p ct", p=P))
                nc.sync.dma_start(out=bp_sb, in_=bp[:].rearrange("(ct p) -> p ct", p=P))
                nc.sync.dma_start(out=mask_sb, in_=mask128[:, :])
                bv_ap = bv[:]
                nc.gpsimd.dma_start(
                    out=bvbc_sb,
                    in_=bass.AP(
                        tensor=bv_ap.tensor, offset=bv_ap.offset, ap=[[0, P]] + list(bv_ap.ap)
                    ),
                )

                # v.T[c, t] = sum_c' Wv[c, c'] x[t, c']  (+ bv[c] per-partition)
                for ts in range(2):
                    pts = [ps1.tile([P, FDT], DT, tag="pts1", name="pts1") for _ in range(NCT)]
                    for ck in range(NCT):
                        for ct in range(NCT):
                            nc.tensor.matmul(
                                pts[ct],
                                wvT_t[:, ck, ct * P : (ct + 1) * P],
                                xT_t[:, ck, ts * FDT : (ts + 1) * FDT],
                                start=(ck == 0),
                                stop=(ck == NCT - 1),
                            )
                    for ct in range(NCT):
                        nc.vector.tensor_scalar_add(
                            vT_sb[:, ct, ts * FDT : (ts + 1) * FDT],
                            pts[ct],
                            bv_sb[:, ct : ct + 1],
                        )

                # v[t, c] = sum_c' x[t, c'] Wv[c, c']  (+ bv[c] broadcast)
                cslices = [(0, FDT), (FDT, C - FDT)]
                for half in range(2):
                    pts2 = [ps1.tile([P, FDT], DT, tag="pts1", name="pts1") for _ in range(8)]
                    for ck in range(NCT):
                        for i in range(4):
                            tt = half * 4 + i
                            for si, (c0, cw) in enumerate(cslices):
                                nc.tensor.matmul(
                                    pts2[i * 2 + si][:, :cw],
                                    xT_t[:, ck, tt * P : (tt + 1) * P],
                                    wvT_t[:, ck, c0 : c0 + cw],
                                    start=(ck == 0),
                                    stop=(ck == NCT - 1),
                                )
                    for i in range(4):
                        tt = half * 4 + i
                        vdst = v_sb[:, tt].rearrange("p (h e) -> p h e", e=HC)
                        for si, (c0, cw) in enumerate(cslices):
                            nh0, nh1 = c0 // HD, (c0 + cw) // HD
                            nc.vector.tensor_add(
                                vdst[:, nh0:nh1, 0:HD],
                                pts2[i * 2 + si][:, :cw].rearrange(
                                    "p (h e) -> p h e", e=HD
                                ),
                                bvbc_sb[:, c0 : c0 + cw].rearrange(
                                    "p (h e) -> p h e", e=HD
                                ),
                            )

            # ---------------- phase 2a: G = Wl @ v ; U.T = (G.T @ (Wc/8).T).T -------
            # G[m, d] = sum_t Wl[m, t] v[t, d]       lhsT = Wl.T[t, m]  rhs = v[t, d]
            # U.T[d, j] = sum_m G[m, d] WcT[m, j]    lhsT = G[m, d]     rhs = WcT[m, j]
            # (the /8 score scale is folded into WcT on the host)
            with (
                tc.tile_pool(name="ph2a", bufs=1) as ph2a,
                tc.tile_pool(name="ps2", bufs=2, space="PSUM") as ps2,
            ):
                wlT_t = ph2a.tile([P, NJT, T], MM_DT)   # Wl.T[t, m]
                wcT_t = ph2a.tile([P, NJT, T], MM_DT)   # Wc.T[m, j] (pre-scaled)
                g_sb = ph2a.tile([P, NJT, C], MM_DT)    # G[m, d] m-tile major
                for mt in range(NJT):
                    nc.sync.dma_start(out=wlT_t[:, mt], in_=WlT[mt * P : (mt + 1) * P, :])
                    nc.sync.dma_start(out=wcT_t[:, mt], in_=WcT[mt * P : (mt + 1) * P, :])
                for ck in range(NCT):
                    nc.sync.dma_start(out=wpT_t[:, ck], in_=WpT[ck * P : (ck + 1) * P, :])

                # G: for each m-tile, accumulate over t; rhs walks v's head columns
                # (strided AP skipping the ones columns), split 512 + 256.
                hslices = [(0, 8), (8, 4)]
                for mt in range(NJT):
                    gp = [
                        ps2.tile([P, nh * HD], DT, tag="gp", name="gp")
                        for _, nh in hslices
                    ]
                    for kt in range(NJT):
                        vv = v_sb[:, kt].rearrange("p (h e) -> p h e", e=HC)
                        for si, (h0, nh) in enumerate(hslices):
                            nc.tensor.matmul(
                                gp[si],
                                wlT_t[:, kt, mt * P : (mt + 1) * P],
                                vv[:, h0 : h0 + nh, 0:HD],
                                start=(kt == 0),
                                stop=(kt == NJT - 1),
                            )
                    for si, (h0, nh) in enumerate(hslices):
                        nc.vector.tensor_copy(
                            g_sb[:, mt, h0 * HD : (h0 + nh) * HD], gp[si]
                        )

                # U.T pair-packed: lhsT = G[m, pair cols], rhs = WcT[m, j slice]
                for hp in range(NPAIR):
                    for js in range(NQS):
                        up = ps2.tile([P, FDT], DT, tag="up", name="up")
                        for mt in range(NJT):
                            nc.tensor.matmul(
                                up,
                                g_sb[:, mt, hp * P : (hp + 1) * P],
                                wcT_t[:, mt, js * FDT : (js + 1) * FDT],
                                start=(mt == 0),
                                stop=(mt == NJT - 1),
                            )
                        nc.vector.tensor_copy(
                            uT_sb[:, hp, js * FDT : (js + 1) * FDT], up
                        )

            # ---------------- phase 2b: scores -> exp -> mask -> PV -> norm --------
            with (
                tc.tile_pool(name="sm", bufs=8) as sm,
                tc.tile_pool(name="p_pool", bufs=3) as p_pool,
                tc.tile_pool(name="outp", bufs=4) as outp,
                tc.tile_pool(name="a_ps", bufs=2, space="PSUM") as a_ps,
                tc.tile_pool(name="y_ps", bufs=2, space="PSUM") as y_ps,
                tc.tile_pool(name="ps3", bufs=2, space="PSUM") as ps3,
            ):
                def emit_proj(ts, cts=range(NCT)):
                    # outT[c_out, t] = Wp @ yT (+bp), ct-outer accumulation chains
                    for ct in cts:
                        pp = ps3.tile([P, FDT], DT, tag="pp")
                        for ck in range(NCT):
                            nc.tensor.matmul(
                                pp,
                                wpT_t[:, ck, ct * P : (ct + 1) * P],
                                yT_sb[:, ck, ts * FDT : (ts + 1) * FDT],
                                start=(ck == 0),
                                stop=(ck == NCT - 1),
                            )
                        ot = outp.tile([P, FDT], DT, tag="ot")
                        nc.vector.tensor_scalar_add(ot, pp, bp_sb[:, ct : ct + 1])
                        nc.sync.dma_start(
                            out=outT[ct * P : (ct + 1) * P, ts * FDT : (ts + 1) * FDT],
                            in_=ot,
                        )

                def pv_gen(hp, qs, pb, jmax):
                    """Generator emitting the PV chains + per-pair normalization;
                    driven interleaved with the NEXT pair's score stream so the
                    PE keeps busy while ACT runs this pair's exps."""
                    q0 = qs * FDT
                    zrec = [
                        sm.tile([1, FDT], MM_DT, tag=f"zrec{hh}", name="zrec")
                        for hh in range(2)
                    ]
                    yu = sm.tile([P, FDT], MM_DT, tag="yu", name="yu", bufs=2)
                    for hh in range(2):
                        h = hp * 2 + hh
                        yp = y_ps.tile([HC, QG], DT, tag="yp", name="yp")
                        for kt in range(jmax + 1):
                            # diagonal k-tiles contribute only to q >= bs; the
                            # masked-out columns never enter the accumulation
                            bs = max(kt - NMSK * qs, 0) * P
                            nc.tensor.matmul(
                                yp[:, bs:FDT],
                                v_sb[:, kt, h * HC : (h + 1) * HC],
                                pb[:, hh, kt, bs:FDT],
                                start=(kt == 0),
                                stop=(kt == jmax),
                            )
                            yield
                        # 1/Z for this head (bf16 row, feeds the broadcast mm);
                        # approx-fast recip off PSUM-staged copy -- a plain
                        # single-partition reciprocal costs ~3.3us on DVE
                        zs = sm.tile([1, FDT], DT, tag="zs", name="zs")
                        zf = sm.tile([1, FDT], DT, tag="zf", name="zf")
                        nc.vector.tensor_copy(zs, yp[HD : HD + 1, :])
                        nc.vector.reciprocal_approx_fast(zf, zs)
                        nc.vector.tensor_copy(zrec[hh], zf)
                        if hh == 0:
                            nc.vector.tensor_copy(yu[0:HD, :], yp[0:HD, :])
                        else:
                            stg = sm.tile([HD, QG], MM_DT, tag="stg", name="stg")
                            nc.vector.tensor_copy(stg, yp[0:HD, :])
                            nc.sync.dma_start(out=yu[HD:P, :], in_=stg)
                        yield
                    # partition-broadcast of 1/Z via two K=1 matmuls, then normalize
                    rbb = ps3.tile([P, FDT], DT, tag="pp", name="rbb")
                    nc.tensor.matmul(
                        rbb[0:HD, :], ones1_sb, zrec[0], start=True, stop=True
                    )
                    nc.tensor.matmul(
                        rbb[HD:P, :],
                        ones1_sb,
                        zrec[1],
                        start=True,
                        stop=True,
                        tile_position=(0, HD),
                    )
                    nc.vector.tensor_mul(yT_sb[:, hp, q0 : q0 + FDT], rbb, yu)
                    yield

                def exhaust(g):
                    if g is not None:
                        for _ in g:
                            pass

                prev_gen = None
                jobs = [(1, hp) for hp in range(NPAIR)] + [(0, hp) for hp in range(NPAIR)]
                for qs, hp in jobs:
                    q0 = qs * FDT
                    jmax = NMSK * qs + NMSK - 1
                    if qs == 0 and hp in (1, 2, 3, 4, 5):
                        i = hp - 1
                        hi = NCT if hp == 5 else i + 1
                        emit_proj(1, range(i, hi))  # big slice's projection as filler
                    # scores: A.T[j, q] single K=64 matmuls, row-packed pairs;
                    # exp on the valid q-range only; mask-mult on the diag band
                    pb = p_pool.tile(
                        [P, 2, NJT, FDT], MM_DT, tag="pb", name="pb"
                    )
                    for jt in range(jmax + 1):
                        jrel = jt - NMSK * qs  # diag position (>=0 on diag tiles)
                        bs = max(jrel, 0) * P  # first valid q column
                        ap2 = a_ps.tile([P, 2, FDT], DT, tag="ap2", name="ap2")
                        for hh in range(2):
                            lo = hh * HD
                            nc.tensor.matmul(
                                ap2[:, hh, :],
                                uT_sb[lo : lo + HD, hp, jt * P : (jt + 1) * P],
                                vT_sb[lo : lo + HD, hp, q0 : q0 + FDT],
                                start=True,
                                stop=True,
                            )
                        nc.scalar.activation(
                            pb[:, :, jt, bs:FDT],
                            ap2[:, :, bs:FDT],
                            mybir.ActivationFunctionType.Exp,
                            bias=bc_sb[:, jt : jt + 1],
                        )
                        if jrel >= 0:  # diagonal tile: mask the 128-wide band
                            for hh in range(2):
                                nc.vector.tensor_mul(
                                    pb[:, hh, jt, bs : bs + P],
                                    pb[:, hh, jt, bs : bs + P],
                                    mask_sb,
                                )
                        if prev_gen is not None:
                            for _ in range(4 if qs == 1 else 6):
                                if next(prev_gen, "end") == "end":
                                    prev_gen = None
                                    break
                    exhaust(prev_gen)
                    prev_gen = pv_gen(hp, qs, pb, jmax)
                exhaust(prev_gen)
                emit_proj(0)

    return nc

_NC = None


def build_nc():
    global _NC
    if _NC is None:
        nc = bacc.Bacc("TRN2", target_bir_lowering=False, debug=False)
        _emit(nc)
        nc.compile()
        _NC = nc
    return _NC


def make_mask128():
    import ml_dtypes

    m = (np.arange(P)[:, None] <= np.arange(P)[None, :]).astype(np.float32)
    return m.astype(ml_dtypes.bfloat16)


def host_inputs(x, Wv, bv, Wl, Wc, bc, Wp, bp):
    """Per-core input maps: layout/dtype prep (transposes + bf16 casts)."""
    import ml_dtypes

    bf16 = ml_dtypes.bfloat16
    x = np.ascontiguousarray(np.asarray(x, dtype=np.float32))
    shared = {
        "WvT": np.ascontiguousarray(np.asarray(Wv, np.float32).T.astype(bf16)),
        "WlT": np.ascontiguousarray(np.asarray(Wl, np.float32).T.astype(bf16)),
        "WcT": np.ascontiguousarray(
            (np.asarray(Wc, np.float32).T / np.sqrt(np.float32(HD))).astype(bf16)
        ),
        "WpT": np.ascontiguousarray(np.asarray(Wp, np.float32).T.astype(bf16)),
        "bv": np.ascontiguousarray(np.asarray(bv, np.float32)),
        "bc": np.ascontiguousarray(np.asarray(bc, np.float32)),
        "bp": np.ascontiguousarray(np.asarray(bp, np.float32)),
        "mask128": make_mask128(),
    }
    return [
        {"xT": np.ascontiguousarray(x[b].T.astype(bf16)), **shared} for b in range(B)
    ]


def kernel(x, Wv, bv, Wl, Wc, bc, Wp, bp, _trace=False):
    global LAST_EXEC_NS, LAST_RESULTS
    nc = build_nc()
    in_maps = host_inputs(x, Wv, bv, Wl, Wc, bc, Wp, bp)
    rb = run_bass_kernel_spmd(nc, in_maps, list(range(B)), trace=_trace)
    LAST_EXEC_NS = rb.exec_time_ns
    LAST_RESULTS = rb
    out = np.stack([rb.results[b]["outT"].T for b in range(B)], axis=0)
    return np.ascontiguousarray(out.astype(np.float32))


# revision 26
# speedup vs baseline: 1.3729x; 1.0661x over previous
"""Trainium2 Bass kernel for CausalSelfAttention with learned TxT score mixing.

Reference computation (per batch b):
    v = x @ Wv.T + bv ; q = k = v
    S = (v_h @ v_h.T) / sqrt(hd)            per head h   [T,T]
    A = S @ Wl.T @ Wc.T + bc                             [T,T]
    P = softmax(causal_mask(A))                          [T,T]
    y_h = P @ v_h ; out = concat(y) @ Wp.T + bp

Key algebra:
  * q == k == v makes S rank-64, so the TxT mixing collapses:
    A.T = Wc @ Wl @ (v_h v_h.T)/8 = ((Wc/8) @ (Wl @ v_h)) @ v_h.T = U_h @ v_h.T.
    We compute G_h = Wl @ v_h (per head, pair-packed) and U_h = (Wc/8) @ G_h
    directly -- no TxT @ TxT precompute at all.  U.T comes out of the second
    matmul in exactly the [d, j] layout the score matmuls need.
  * Scores stay in [key, query] layout; softmax uses unnormalized exp() and
    the normalizer Z[q] falls out of the PV matmul via a ones-column in the
    stationary operand (logits are O(1); masked entries are exactly zero).
  * Causal structure at 128 granularity: exp() runs only on the valid
    q-range of each key tile, the 0/1 mask multiply touches only the 128-wide
    diagonal band, and the fully-masked region is memset to zero.
  * Per-pair normalization: 1/Z rows are partition-broadcast with a tiny
    K=2 matmul (no DRAM round-trips), so the tail of the kernel is short.
  * Matmul operands are bf16 (accumulation, softmax and bias math in fp32);
    head pairs are row/col-packed into the 128-wide PE array; per-pair PV
    chains are software-pipelined against the next pair's score/exp stream.

Sharding: data-parallel over batch (core b <- batch b). All weights
replicated; host passes transposed bf16 copies (layout/dtype prep only).
"""

import os
import sys

for _p in ("/opt/trn_rl_repo", "/root/.axon_site/_ro/trn_rl_repo"):
    if os.path.isdir(_p) and _p not in sys.path:
        sys.path.insert(0, _p)

import numpy as np

import concourse.bass as bass
import concourse.tile as tile
from concourse import bacc, mybir
from concourse.bass_utils import run_bass_kernel_spmd

B, T, C, NH = 8, 1024, 768, 12
HD = C // NH          # 64
P = 128               # SBUF partitions
NJT = T // P          # 8 j/k tiles of 128
NCT = C // P          # 6 c tiles of 128
NPAIR = NH // 2       # 6 head pairs (two heads share a 128-partition tile)
QG = 512              # q granularity for scores/exp/PV (= q slice width)
NQS = T // QG         # 2
FDT = QG
NMSK = QG // P        # 4 diagonal-tile positions per q slice
HC = HD + 1           # 65: head value columns + ones column for Z
DT = mybir.dt.float32
BF = mybir.dt.bfloat16
MM_DT = BF            # dtype of all matmul operands (accumulation stays fp32)

LAST_EXEC_NS = None
LAST_RESULTS = None


def _emit(nc):
    """Emit the whole per-core program inside a TileContext."""
    xT = nc.dram_tensor("xT", [C, T], MM_DT, kind="ExternalInput")
    WvT = nc.dram_tensor("WvT", [C, C], MM_DT, kind="ExternalInput")
    WlT = nc.dram_tensor("WlT", [T, T], MM_DT, kind="ExternalInput")
    WcT = nc.dram_tensor("WcT", [T, T], MM_DT, kind="ExternalInput")  # pre-scaled /8
    WpT = nc.dram_tensor("WpT", [C, C], MM_DT, kind="ExternalInput")
    bv = nc.dram_tensor("bv", [C], DT, kind="ExternalInput")
    bc = nc.dram_tensor("bc", [T], DT, kind="ExternalInput")
    bp = nc.dram_tensor("bp", [C], DT, kind="ExternalInput")
    mask128 = nc.dram_tensor("mask128", [P, P], MM_DT, kind="ExternalInput")
    outT = nc.dram_tensor("outT", [C, T], DT, kind="ExternalOutput")

    with tile.TileContext(nc) as tc:
        with tc.tile_pool(name="consts", bufs=1) as consts:
            # Flat allocation: no SBUF reuse between phases. Pool-level reuse
            # creates anti-dependencies that serialize the next phase's input
            # DMAs behind the previous phase's matmul reads.
            vT_sb = consts.tile([P, NCT, T], MM_DT)          # v.T  [c, t]
            v_sb = consts.tile([P, NJT, NH * HC], MM_DT)     # v    [t, h*65+d], col 64 = ones
            uT_sb = consts.tile([P, NPAIR, T], MM_DT)        # U.T pair-packed [hh*64+d, hp, j]
            yT_sb = consts.tile([P, NCT, T], MM_DT)          # normalized attn out, [c, t]
            wpT_t = consts.tile([P, NCT, C], MM_DT)          # Wp.T[c, c_out]
            wlT_t = consts.tile([P, NJT, T], MM_DT)          # Wl.T[t, m]
            wcT_t = consts.tile([P, NJT, T], MM_DT)          # Wc.T[m, j] (pre-scaled)
            g_sb = consts.tile([P, NJT, C], MM_DT)           # G[m, d] m-tile major
            mask_sb = consts.tile([P, P], MM_DT)             # lower-tri [jl, qq]: jl<=qq
            ones1_sb = consts.tile([1, HD], MM_DT)           # K=1 broadcast stationary
            bc_sb = consts.tile([P, NJT], DT)
            bv_sb = consts.tile([P, NCT], DT)
            bvbc_sb = consts.tile([P, C], DT)                # bv broadcast across partitions
            bp_sb = consts.tile([P, NCT], DT)

            # xT/wvT live in a scoped pool: they are only read until mid-kernel,
            # so later pools may safely reuse their space (the anti-dependency
            # resolves long before those pools' first writes).
            ph1_cm = tc.tile_pool(name="ph1", bufs=1)
            ph1 = ph1_cm.__enter__()
            xT_t = ph1.tile([P, NCT, T], MM_DT)
            wvT_t = ph1.tile([P, NCT, C], MM_DT)

            # Input DMAs fan out over three engine queues so the phase-1
            # operands (wvT + xT) land first and in parallel.
            for ck in range(NCT):
                nc.sync.dma_start(out=wvT_t[:, ck], in_=WvT[ck * P : (ck + 1) * P, :])
                nc.gpsimd.dma_start(out=xT_t[:, ck], in_=xT[ck * P : (ck + 1) * P, :])
            nc.scalar.dma_start(out=bc_sb, in_=bc[:].rearrange("(jt p) -> p jt", p=P))
            nc.scalar.dma_start(out=bv_sb, in_=bv[:].rearrange("(ct p) -> p ct", p=P))
            nc.scalar.dma_start(out=bp_sb, in_=bp[:].rearrange("(ct p) -> p ct", p=P))
            nc.scalar.dma_start(out=mask_sb, in_=mask128[:, :])
            bv_ap = bv[:]
            nc.scalar.dma_start(
                out=bvbc_sb,
                in_=bass.AP(
                    tensor=bv_ap.tensor, offset=bv_ap.offset, ap=[[0, P]] + list(bv_ap.ap)
                ),
            )
            for mt in range(NJT):
                nc.sync.dma_start(out=wlT_t[:, mt], in_=WlT[mt * P : (mt + 1) * P, :])
                nc.gpsimd.dma_start(out=wcT_t[:, mt], in_=WcT[mt * P : (mt + 1) * P, :])
            for ck in range(NCT):
                nc.sync.dma_start(out=wpT_t[:, ck], in_=WpT[ck * P : (ck + 1) * P, :])

            # ones columns of v_sb (column h*65+64 <- 1.0), for the Z row of PV
            for tt in range(NJT):
                nc.vector.memset(
                    v_sb[:, tt].rearrange("p (h e) -> p h e", e=HC)[:, :, HD : HD + 1],
                    1.0,
                )
            nc.vector.memset(ones1_sb, 1.0)

            # ---------------- phase 1: v (natural) and v.T projections ------------
            with tc.tile_pool(name="ps1", bufs=8, space="PSUM") as ps1:
                # v[t, c] = sum_c' x[t, c'] Wv[c, c']  (+ bv[c] broadcast)
                # (natural layout first: its PSUM->SBUF adds drain under the
                # following vT matmul stream, so phase 2a starts stall-free)
                cslices = [(0, FDT), (FDT, C - FDT)]
                for half in range(2):
                    pts2 = [ps1.tile([P, FDT], DT, tag="pts1", name="pts1") for _ in range(8)]
                    for ck in range(NCT):
                        for i in range(4):
                            tt = half * 4 + i
                            for si, (c0, cw) in enumerate(cslices):
                                nc.tensor.matmul(
                                    pts2[i * 2 + si][:, :cw],
                                    xT_t[:, ck, tt * P : (tt + 1) * P],
                                    wvT_t[:, ck, c0 : c0 + cw],
                                    start=(ck == 0),
                                    stop=(ck == NCT - 1),
                                )
                    for i in range(4):
                        tt = half * 4 + i
                        vdst = v_sb[:, tt].rearrange("p (h e) -> p h e", e=HC)
                        for si, (c0, cw) in enumerate(cslices):
                            nh0, nh1 = c0 // HD, (c0 + cw) // HD
                            nc.vector.tensor_add(
                                vdst[:, nh0:nh1, 0:HD],
                                pts2[i * 2 + si][:, :cw].rearrange(
                                    "p (h e) -> p h e", e=HD
                                ),
                                bvbc_sb[:, c0 : c0 + cw].rearrange(
                                    "p (h e) -> p h e", e=HD
                                ),
                            )

                # v.T[c, t] = sum_c' Wv[c, c'] x[t, c']  (+ bv[c] per-partition)
                for ts in range(2):
                    pts = [ps1.tile([P, FDT], DT, tag="pts1", name="pts1") for _ in range(NCT)]
                    for ck in range(NCT):
                        for ct in range(NCT):
                            nc.tensor.matmul(
                                pts[ct],
                                wvT_t[:, ck, ct * P : (ct + 1) * P],
                                xT_t[:, ck, ts * FDT : (ts + 1) * FDT],
                                start=(ck == 0),
                                stop=(ck == NCT - 1),
                            )
                    for ct in range(NCT):
                        nc.vector.tensor_scalar_add(
                            vT_sb[:, ct, ts * FDT : (ts + 1) * FDT],
                            pts[ct],
                            bv_sb[:, ct : ct + 1],
                        )
            ph1_cm.__exit__(None, None, None)

            # ---------------- phase 2a: G = Wl @ v ; U.T = (G.T @ (Wc/8).T).T -------
            # G[m, d] = sum_t Wl[m, t] v[t, d]       lhsT = Wl.T[t, m]  rhs = v[t, d]
            # U.T[d, j] = sum_m G[m, d] WcT[m, j]    lhsT = G[m, d]     rhs = WcT[m, j]
            # (the /8 score scale is folded into WcT on the host)
            with tc.tile_pool(name="ps2", bufs=2, space="PSUM") as ps2:
                # G: for each m-tile, accumulate over t; rhs walks v's head columns
                # (strided AP skipping the ones columns), split 512 + 256.
                hslices = [(0, 8), (8, 4)]
                for mt in range(NJT):
                    gp = [
                        ps2.tile([P, nh * HD], DT, tag=f"gp{si}", name="gp")
                        for si, (_, nh) in enumerate(hslices)
                    ]
                    for kt in range(NJT):
                        vv = v_sb[:, kt].rearrange("p (h e) -> p h e", e=HC)
                        for si, (h0, nh) in enumerate(hslices):
                            nc.tensor.matmul(
                                gp[si],
                                wlT_t[:, kt, mt * P : (mt + 1) * P],
                                vv[:, h0 : h0 + nh, 0:HD],
                                start=(kt == 0),
                                stop=(kt == NJT - 1),
                            )
                    for si, (h0, nh) in enumerate(hslices):
                        nc.vector.tensor_copy(
                            g_sb[:, mt, h0 * HD : (h0 + nh) * HD], gp[si]
                        )

                # U.T pair-packed: lhsT = G[m, pair cols], rhs = WcT[m, j slice]
                for hp in range(NPAIR):
                    for js in range(NQS):
                        up = ps2.tile([P, FDT], DT, tag="up", name="up")
                        for mt in range(NJT):
                            nc.tensor.matmul(
                                up,
                                g_sb[:, mt, hp * P : (hp + 1) * P],
                                wcT_t[:, mt, js * FDT : (js + 1) * FDT],
                                start=(mt == 0),
                                stop=(mt == NJT - 1),
                            )
                        nc.vector.tensor_copy(
                            uT_sb[:, hp, js * FDT : (js + 1) * FDT], up
                        )

            # ---------------- phase 2b: scores -> exp -> mask -> PV -> norm --------
            with (
                tc.tile_pool(name="sm", bufs=2) as sm,
                tc.tile_pool(name="p_pool", bufs=3) as p_pool,
                tc.tile_pool(name="outp", bufs=4) as outp,
                tc.tile_pool(name="a_ps", bufs=2, space="PSUM") as a_ps,
                tc.tile_pool(name="y_ps", bufs=2, space="PSUM") as y_ps,
                tc.tile_pool(name="ps3", bufs=2, space="PSUM") as ps3,
            ):
                def emit_proj(ts, cts=range(NCT)):
                    # outT[c_out, t] = Wp @ yT (+bp), ct-outer accumulation chains
                    for ct in cts:
                        pp = ps3.tile([P, FDT], DT, tag="pp")
                        for ck in range(NCT):
                            nc.tensor.matmul(
                                pp,
                                wpT_t[:, ck, ct * P : (ct + 1) * P],
                                yT_sb[:, ck, ts * FDT : (ts + 1) * FDT],
                                start=(ck == 0),
                                stop=(ck == NCT - 1),
                            )
                        ot = outp.tile([P, FDT], DT, tag="ot")
                        nc.vector.tensor_scalar_add(ot, pp, bp_sb[:, ct : ct + 1])
                        nc.sync.dma_start(
                            out=outT[ct * P : (ct + 1) * P, ts * FDT : (ts + 1) * FDT],
                            in_=ot,
                        )

                def pv_gen(hp, qs, pb, jmax):
                    """Generator emitting the PV chains + per-pair normalization;
                    driven interleaved with the NEXT pair's score stream so the
                    PE keeps busy while ACT runs this pair's exps."""
                    q0 = qs * FDT
                    zrec = [
                        sm.tile([1, FDT], MM_DT, tag=f"zrec{hh}", name="zrec")
                        for hh in range(2)
                    ]
                    yu = sm.tile([P, FDT], MM_DT, tag="yu", name="yu", bufs=2)
                    for hh in range(2):
                        h = hp * 2 + hh
                        yp = y_ps.tile([HC, QG], DT, tag="yp", name="yp")
                        for kt in range(jmax + 1):
                            # diagonal k-tiles contribute only to q >= bs; the
                            # masked-out columns never enter the accumulation
                            bs = max(kt - NMSK * qs, 0) * P
                            nc.tensor.matmul(
                                yp[:, bs:FDT],
                                v_sb[:, kt, h * HC : (h + 1) * HC],
                                pb[:, hh, kt, bs:FDT],
                                start=(kt == 0),
                                stop=(kt == jmax),
                            )
                            yield
                        # 1/Z for this head (bf16 row, feeds the broadcast mm);
                        # approx-fast recip off PSUM-staged copy -- a plain
                        # single-partition reciprocal costs ~3.3us on DVE
                        zs = sm.tile([1, FDT], DT, tag="zs", name="zs")
                        zf = sm.tile([1, FDT], DT, tag="zf", name="zf")
                        nc.vector.tensor_copy(zs, yp[HD : HD + 1, :])
                        nc.vector.reciprocal_approx_fast(zf, zs)
                        nc.vector.tensor_copy(zrec[hh], zf)
                        if hh == 0:
                            nc.vector.tensor_copy(yu[0:HD, :], yp[0:HD, :])
                        else:
                            stg = sm.tile([HD, QG], MM_DT, tag="stg", name="stg")
                            nc.vector.tensor_copy(stg, yp[0:HD, :])
                            nc.sync.dma_start(out=yu[HD:P, :], in_=stg)
                        yield
                    # partition-broadcast of 1/Z via two K=1 matmuls, then normalize
                    rbb = ps3.tile([P, FDT], DT, tag="pp", name="rbb")
                    nc.tensor.matmul(
                        rbb[0:HD, :], ones1_sb, zrec[0], start=True, stop=True
                    )
                    nc.tensor.matmul(
                        rbb[HD:P, :],
                        ones1_sb,
                        zrec[1],
                        start=True,
                        stop=True,
                        tile_position=(0, HD),
                    )
                    nc.vector.tensor_mul(yT_sb[:, hp, q0 : q0 + FDT], rbb, yu)
                    yield

                def exhaust(g):
                    if g is not None:
                        for _ in g:
                            pass

                prev_gen = None
                jobs = [(1, hp) for hp in range(NPAIR)] + [(0, hp) for hp in range(NPAIR)]
                for qs, hp in jobs:
                    q0 = qs * FDT
                    jmax = NMSK * qs + NMSK - 1
                    if qs == 0 and hp in (1, 2, 3, 4, 5):
                        i = hp - 1
                        hi = NCT if hp == 5 else i + 1
                        emit_proj(1, range(i, hi))  # big slice's projection as filler
                    # scores: A.T[j, q] single K=64 matmuls, row-packed pairs;
                    # exp on the valid q-range only; mask-mult on the diag band
                    pb = p_pool.tile(
                        [P, 2, NJT, FDT], MM_DT, tag="pb", name="pb"
                    )
                    for jt in range(jmax + 1):
                        jrel = jt - NMSK * qs  # diag position (>=0 on diag tiles)
                        bs = max(jrel, 0) * P  # first valid q column
                        ap2 = a_ps.tile([P, 2, FDT], DT, tag="ap2", name="ap2")
                        for hh in range(2):
                            lo = hh * HD
                            nc.tensor.matmul(
                                ap2[:, hh, :],
                                uT_sb[lo : lo + HD, hp, jt * P : (jt + 1) * P],
                                vT_sb[lo : lo + HD, hp, q0 : q0 + FDT],
                                start=True,
                                stop=True,
                            )
                        nc.scalar.activation(
                            pb[:, :, jt, bs:FDT],
                            ap2[:, :, bs:FDT],
                            mybir.ActivationFunctionType.Exp,
                            bias=bc_sb[:, jt : jt + 1],
                        )
                        if jrel >= 0:  # diagonal tile: mask the 128-wide band
                            for hh in range(2):
                                nc.vector.tensor_mul(
                                    pb[:, hh, jt, bs : bs + P],
                                    pb[:, hh, jt, bs : bs + P],
                                    mask_sb,
                                )
                        if prev_gen is not None:
                            for _ in range(4 if qs == 1 else 6):
                                if next(prev_gen, "end") == "end":
                                    prev_gen = None
                                    break
                    exhaust(prev_gen)
                    prev_gen = pv_gen(hp, qs, pb, jmax)
                exhaust(prev_gen)
                emit_proj(0)

    return nc

_NC = None


def build_nc():
    global _NC
    if _NC is None:
        nc = bacc.Bacc("TRN2", target_bir_lowering=False, debug=False)
        _emit(nc)
        nc.compile()
        _NC = nc
    return _NC


def make_mask128():
    import ml_dtypes

    m = (np.arange(P)[:, None] <= np.arange(P)[None, :]).astype(np.float32)
    return m.astype(ml_dtypes.bfloat16)


def host_inputs(x, Wv, bv, Wl, Wc, bc, Wp, bp):
    """Per-core input maps: layout/dtype prep (transposes + bf16 casts)."""
    import ml_dtypes

    bf16 = ml_dtypes.bfloat16
    x = np.ascontiguousarray(np.asarray(x, dtype=np.float32))
    shared = {
        "WvT": np.ascontiguousarray(np.asarray(Wv, np.float32).T.astype(bf16)),
        "WlT": np.ascontiguousarray(np.asarray(Wl, np.float32).T.astype(bf16)),
        "WcT": np.ascontiguousarray(
            (np.asarray(Wc, np.float32).T / np.sqrt(np.float32(HD))).astype(bf16)
        ),
        "WpT": np.ascontiguousarray(np.asarray(Wp, np.float32).T.astype(bf16)),
        "bv": np.ascontiguousarray(np.asarray(bv, np.float32)),
        "bc": np.ascontiguousarray(np.asarray(bc, np.float32)),
        "bp": np.ascontiguousarray(np.asarray(bp, np.float32)),
        "mask128": make_mask128(),
    }
    return [
        {"xT": np.ascontiguousarray(x[b].T.astype(bf16)), **shared} for b in range(B)
    ]


def kernel(x, Wv, bv, Wl, Wc, bc, Wp, bp, _trace=False):
    global LAST_EXEC_NS, LAST_RESULTS
    nc = build_nc()
    in_maps = host_inputs(x, Wv, bv, Wl, Wc, bc, Wp, bp)
    rb = run_bass_kernel_spmd(nc, in_maps, list(range(B)), trace=_trace)
    LAST_EXEC_NS = rb.exec_time_ns
    LAST_RESULTS = rb
    out = np.stack([rb.results[b]["outT"].T for b in range(B)], axis=0)
    return np.ascontiguousarray(out.astype(np.float32))


# revision 30
# speedup vs baseline: 1.4409x; 1.0495x over previous
"""Trainium2 Bass kernel for CausalSelfAttention with learned TxT score mixing.

Reference computation (per batch b):
    v = x @ Wv.T + bv ; q = k = v
    S = (v_h @ v_h.T) / sqrt(hd)            per head h   [T,T]
    A = S @ Wl.T @ Wc.T + bc                             [T,T]
    P = softmax(causal_mask(A))                          [T,T]
    y_h = P @ v_h ; out = concat(y) @ Wp.T + bp

Key algebra:
  * q == k == v makes S rank-64, so the TxT mixing collapses:
    A.T = Wc @ Wl @ (v_h v_h.T)/8 = ((Wc/8) @ (Wl @ v_h)) @ v_h.T = U_h @ v_h.T.
    We compute G_h = Wl @ v_h (per head, pair-packed) and U_h = (Wc/8) @ G_h
    directly -- no TxT @ TxT precompute at all.  U.T comes out of the second
    matmul in exactly the [d, j] layout the score matmuls need.
  * Scores stay in [key, query] layout; softmax uses unnormalized exp() and
    the normalizer Z[q] falls out of the PV matmul via a ones-column in the
    stationary operand (logits are O(1); masked entries are exactly zero).
  * Causal structure at 128 granularity: exp() runs only on the valid
    q-range of each key tile, the 0/1 mask multiply touches only the 128-wide
    diagonal band, and diagonal PV matmuls accumulate into the valid
    column sub-range only (the masked region never enters the sum).
  * Per-pair normalization: 1/Z rows are partition-broadcast with two K=1
    matmuls (col tile_position), no DRAM round-trips, so the tail is short.
  * Scheduling notes: dependencies are tile-granular, so DMA'd operands are
    split into per-slab tiles and U.T into per-(pair, j-half) tiles; U.T
    chains are emitted inside the job loop so the scalar engine's exp stream
    overlaps them; engine queues are in-order, so PSUM->SBUF drains are
    interleaved chain-by-chain with the matmul stream.

Sharding: data-parallel over batch (core b <- batch b). All weights
replicated; host passes transposed bf16 copies (layout/dtype prep only).
"""

import os
import sys

for _p in ("/opt/trn_rl_repo", "/root/.axon_site/_ro/trn_rl_repo"):
    if os.path.isdir(_p) and _p not in sys.path:
        sys.path.insert(0, _p)

import numpy as np

import concourse.bass as bass
import concourse.tile as tile
from concourse import bacc, mybir
from concourse.bass_utils import run_bass_kernel_spmd

B, T, C, NH = 8, 1024, 768, 12
HD = C // NH          # 64
P = 128               # SBUF partitions
NJT = T // P          # 8 j/k tiles of 128
NCT = C // P          # 6 c tiles of 128
NPAIR = NH // 2       # 6 head pairs (two heads share a 128-partition tile)
QG = 512              # q granularity for scores/exp/PV (= q slice width)
NQS = T // QG         # 2
FDT = QG
NMSK = QG // P        # 4 diagonal-tile positions per q slice
HC = HD + 1           # 65: head value columns + ones column for Z
DT = mybir.dt.float32
BF = mybir.dt.bfloat16
MM_DT = BF            # dtype of all matmul operands (accumulation stays fp32)

LAST_EXEC_NS = None
LAST_RESULTS = None


def _emit(nc):
    """Emit the whole per-core program inside a TileContext."""
    xT = nc.dram_tensor("xT", [C, T], MM_DT, kind="ExternalInput")
    WvT = nc.dram_tensor("WvT", [C, C], MM_DT, kind="ExternalInput")
    WlT = nc.dram_tensor("WlT", [T, T], MM_DT, kind="ExternalInput")
    WcT = nc.dram_tensor("WcT", [T, T], MM_DT, kind="ExternalInput")  # pre-scaled /8
    WpT = nc.dram_tensor("WpT", [C, C], MM_DT, kind="ExternalInput")
    bv = nc.dram_tensor("bv", [C], DT, kind="ExternalInput")
    bc = nc.dram_tensor("bc", [T], DT, kind="ExternalInput")
    bp = nc.dram_tensor("bp", [C], DT, kind="ExternalInput")
    mask128 = nc.dram_tensor("mask128", [P, P], MM_DT, kind="ExternalInput")
    outT = nc.dram_tensor("outT", [C, T], DT, kind="ExternalOutput")

    with tile.TileContext(nc) as tc:
        with tc.tile_pool(name="consts", bufs=1) as consts:
            vT_sb = consts.tile([P, NCT, T], MM_DT)          # v.T  [c, t]
            v_sb = consts.tile([P, NJT, NH * HC], MM_DT)     # v    [t, h*65+d], col 64 = ones
            # U.T pair-packed [hh*64+d, j], split per (pair, j-half) so each
            # score matmul depends on exactly one chain's copy
            uT_t = [
                [
                    consts.tile([P, FDT], MM_DT, name=f"uT_{hp}_{js}")
                    for js in range(NQS)
                ]
                for hp in range(NPAIR)
            ]
            yT_sb = consts.tile([P, NCT, T], MM_DT)          # normalized attn out, [c, t]
            wpT_t = consts.tile([P, NCT, C], MM_DT)          # Wp.T[c, c_out]
            wlT_t = consts.tile([P, NJT, T], MM_DT)          # Wl.T[t, m]
            wcT_t = consts.tile([P, NJT, T], MM_DT)          # Wc.T[m, j] (pre-scaled)
            g_sb = consts.tile([P, NJT, C], MM_DT)           # G[m, d] m-tile major
            mask_sb = consts.tile([P, P], MM_DT)             # lower-tri [jl, qq]: jl<=qq
            ones1_sb = consts.tile([1, HD], MM_DT)           # K=1 broadcast stationary
            bc_sb = consts.tile([P, NJT], DT)
            bv_sb = consts.tile([P, NCT], DT)
            bvbc_sb = consts.tile([P, C], DT)                # bv broadcast across partitions
            bp_sb = consts.tile([P, NCT], DT)

            # xT/wvT live in a scoped pool (read only until mid-kernel; later
            # pools may reuse the space -- that anti-dependency resolves long
            # before their first writes).  Per-slab tiles keep deps precise.
            ph1_cm = tc.tile_pool(name="ph1", bufs=1)
            ph1 = ph1_cm.__enter__()
            xT_t = [ph1.tile([P, T], MM_DT, name=f"xT_{ck}") for ck in range(NCT)]
            wvT_t = [ph1.tile([P, C], MM_DT, name=f"wvT_{ck}") for ck in range(NCT)]

            # Input DMAs fan out over three engine queues so the phase-1
            # operands (wvT + xT) land first and in parallel.
            for ck in range(NCT):
                nc.sync.dma_start(out=wvT_t[ck], in_=WvT[ck * P : (ck + 1) * P, :])
                nc.gpsimd.dma_start(out=xT_t[ck], in_=xT[ck * P : (ck + 1) * P, :])
            nc.scalar.dma_start(out=bc_sb, in_=bc[:].rearrange("(jt p) -> p jt", p=P))
            nc.scalar.dma_start(out=bv_sb, in_=bv[:].rearrange("(ct p) -> p ct", p=P))
            nc.scalar.dma_start(out=bp_sb, in_=bp[:].rearrange("(ct p) -> p ct", p=P))
            nc.scalar.dma_start(out=mask_sb, in_=mask128[:, :])
            bv_ap = bv[:]
            nc.scalar.dma_start(
                out=bvbc_sb,
                in_=bass.AP(
                    tensor=bv_ap.tensor, offset=bv_ap.offset, ap=[[0, P]] + list(bv_ap.ap)
                ),
            )
            for mt in range(NJT):
                nc.sync.dma_start(out=wlT_t[:, mt], in_=WlT[mt * P : (mt + 1) * P, :])
                nc.gpsimd.dma_start(out=wcT_t[:, mt], in_=WcT[mt * P : (mt + 1) * P, :])
            for ck in range(NCT):
                nc.sync.dma_start(out=wpT_t[:, ck], in_=WpT[ck * P : (ck + 1) * P, :])

            # ones columns of v_sb (column h*65+64 <- 1.0), for the Z row of PV
            for tt in range(NJT):
                nc.vector.memset(
                    v_sb[:, tt].rearrange("p (h e) -> p h e", e=HC)[:, :, HD : HD + 1],
                    1.0,
                )
            nc.vector.memset(ones1_sb, 1.0)

            # ---------------- phase 1: v (natural) and v.T projections ------------
            # chain-contiguous: each PSUM tile's 6-matmul chain completes before
            # the next begins, so the PSUM->SBUF bias-add drains interleave with
            # the matmul stream instead of bunching at phase end.
            with tc.tile_pool(name="ps1", bufs=8, space="PSUM") as ps1:
                # v[t, c] = sum_c' x[t, c'] Wv[c, c']  (+ bv[c] broadcast)
                cslices = [(0, FDT), (FDT, C - FDT)]
                for tt in range(NJT):
                    for si, (c0, cw) in enumerate(cslices):
                        pt = ps1.tile([P, FDT], DT, tag="pts1", name="pts1")
                        for ck in range(NCT):
                            nc.tensor.matmul(
                                pt[:, :cw],
                                xT_t[ck][:, tt * P : (tt + 1) * P],
                                wvT_t[ck][:, c0 : c0 + cw],
                                start=(ck == 0),
                                stop=(ck == NCT - 1),
                            )
                        vdst = v_sb[:, tt].rearrange("p (h e) -> p h e", e=HC)
                        nh0, nh1 = c0 // HD, (c0 + cw) // HD
                        nc.vector.tensor_add(
                            vdst[:, nh0:nh1, 0:HD],
                            pt[:, :cw].rearrange("p (h e) -> p h e", e=HD),
                            bvbc_sb[:, c0 : c0 + cw].rearrange("p (h e) -> p h e", e=HD),
                        )

                # v.T[c, t] = sum_c' Wv[c, c'] x[t, c']  (+ bv[c] per-partition)
                for ts in range(2):
                    for ct in range(NCT):
                        pt = ps1.tile([P, FDT], DT, tag="pts1", name="pts1")
                        for ck in range(NCT):
                            nc.tensor.matmul(
                                pt,
                                wvT_t[ck][:, ct * P : (ct + 1) * P],
                                xT_t[ck][:, ts * FDT : (ts + 1) * FDT],
                                start=(ck == 0),
                                stop=(ck == NCT - 1),
                            )
                        nc.vector.tensor_scalar_add(
                            vT_sb[:, ct, ts * FDT : (ts + 1) * FDT],
                            pt,
                            bv_sb[:, ct : ct + 1],
                        )
            ph1_cm.__exit__(None, None, None)

            # ---------------- phase 2a: G = Wl @ v ------------------------------
            # G[m, d] = sum_t Wl[m, t] v[t, d]       lhsT = Wl.T[t, m]  rhs = v[t, d]
            # (rhs walks v's head columns with a strided AP skipping the ones
            # columns, split 512 + 256 to fit PSUM banks)
            hslices = [(0, 8), (8, 4)]
            with tc.tile_pool(name="ps2", bufs=2, space="PSUM") as ps2:
                for mt in range(NJT):
                    gp = [
                        ps2.tile([P, nh * HD], DT, tag=f"gp{si}", name="gp")
                        for si, (_, nh) in enumerate(hslices)
                    ]
                    for kt in range(NJT):
                        vv = v_sb[:, kt].rearrange("p (h e) -> p h e", e=HC)
                        for si, (h0, nh) in enumerate(hslices):
                            nc.tensor.matmul(
                                gp[si],
                                wlT_t[:, kt, mt * P : (mt + 1) * P],
                                vv[:, h0 : h0 + nh, 0:HD],
                                start=(kt == 0),
                                stop=(kt == NJT - 1),
                            )
                    for si, (h0, nh) in enumerate(hslices):
                        nc.vector.tensor_copy(
                            g_sb[:, mt, h0 * HD : (h0 + nh) * HD], gp[si]
                        )

            # ---------------- phase 2b: U.T / scores / exp / mask / PV / norm ----
            # U.T[d, j] = sum_m G[m, d] WcT[m, j]    lhsT = G[m, d]   rhs = WcT[m, j]
            # U.T chains are emitted per-pair inside the job loop, so the exp
            # stream (ACT) of earlier pairs overlaps later pairs' U.T matmuls.
            with (
                tc.tile_pool(name="sm", bufs=2) as sm,
                tc.tile_pool(name="p_pool", bufs=3) as p_pool,
                tc.tile_pool(name="outp", bufs=4) as outp,
                tc.tile_pool(name="a_ps", bufs=2, space="PSUM") as a_ps,
                tc.tile_pool(name="y_ps", bufs=2, space="PSUM") as y_ps,
                tc.tile_pool(name="ps3", bufs=2, space="PSUM") as ps3,
            ):
                def emit_ut(hp):
                    for js in range(NQS):  # js=0 first: q-slice-0 tiles ready first
                        up = ps3.tile([P, FDT], DT, tag="pp", name="up")
                        for mt in range(NJT):
                            nc.tensor.matmul(
                                up,
                                g_sb[:, mt, hp * P : (hp + 1) * P],
                                wcT_t[:, mt, js * FDT : (js + 1) * FDT],
                                start=(mt == 0),
                                stop=(mt == NJT - 1),
                            )
                        nc.vector.tensor_copy(uT_t[hp][js], up)

                def emit_proj(ts, cts=range(NCT)):
                    # outT[c_out, t] = Wp @ yT (+bp), ct-outer accumulation chains
                    for ct in cts:
                        pp = ps3.tile([P, FDT], DT, tag="pp")
                        for ck in range(NCT):
                            nc.tensor.matmul(
                                pp,
                                wpT_t[:, ck, ct * P : (ct + 1) * P],
                                yT_sb[:, ck, ts * FDT : (ts + 1) * FDT],
                                start=(ck == 0),
                                stop=(ck == NCT - 1),
                            )
                        ot = outp.tile([P, FDT], DT, tag="ot")
                        nc.vector.tensor_scalar_add(ot, pp, bp_sb[:, ct : ct + 1])
                        nc.gpsimd.dma_start(
                            out=outT[ct * P : (ct + 1) * P, ts * FDT : (ts + 1) * FDT],
                            in_=ot,
                        )

                def pv_gen(hp, qs, pb, jmax):
                    """Generator emitting the PV chains + per-pair normalization;
                    driven interleaved with the NEXT pair's score stream so the
                    PE keeps busy while ACT runs this pair's exps."""
                    q0 = qs * FDT
                    zrec = [
                        sm.tile([1, FDT], MM_DT, tag=f"zrec{hh}", name="zrec")
                        for hh in range(2)
                    ]
                    yu = sm.tile([P, FDT], MM_DT, tag="yu", name="yu", bufs=2)
                    for hh in range(2):
                        h = hp * 2 + hh
                        yp = y_ps.tile([HC, QG], DT, tag="yp", name="yp")
                        for kt in range(jmax + 1):
                            # diagonal k-tiles contribute only to q >= bs; the
                            # masked-out columns never enter the accumulation
                            bs = max(kt - NMSK * qs, 0) * P
                            nc.tensor.matmul(
                                yp[:, bs:FDT],
                                v_sb[:, kt, h * HC : (h + 1) * HC],
                                pb[:, hh, kt, bs:FDT],
                                start=(kt == 0),
                                stop=(kt == jmax),
                            )
                            yield
                        # 1/Z for this head (bf16 row, feeds the broadcast mm);
                        # approx-fast recip off a PSUM-staged copy -- a plain
                        # single-partition reciprocal costs ~3.3us on DVE
                        zs = sm.tile([1, FDT], DT, tag="zs", name="zs")
                        zf = sm.tile([1, FDT], DT, tag="zf", name="zf")
                        nc.vector.tensor_copy(zs, yp[HD : HD + 1, :])
                        nc.vector.reciprocal_approx_fast(zf, zs)
                        nc.vector.tensor_copy(zrec[hh], zf)
                        if hh == 0:
                            nc.vector.tensor_copy(yu[0:HD, :], yp[0:HD, :])
                        else:
                            stg = sm.tile([HD, QG], MM_DT, tag="stg", name="stg")
                            nc.vector.tensor_copy(stg, yp[0:HD, :])
                            nc.sync.dma_start(out=yu[HD:P, :], in_=stg)
                        yield
                    # partition-broadcast of 1/Z via two K=1 matmuls, then normalize
                    rbb = ps3.tile([P, FDT], DT, tag="pp", name="rbb")
                    nc.tensor.matmul(
                        rbb[0:HD, :], ones1_sb, zrec[0], start=True, stop=True
                    )
                    nc.tensor.matmul(
                        rbb[HD:P, :],
                        ones1_sb,
                        zrec[1],
                        start=True,
                        stop=True,
                        tile_position=(0, HD),
                    )
                    nc.vector.tensor_mul(yT_sb[:, hp, q0 : q0 + FDT], rbb, yu)
                    yield

                def exhaust(g):
                    if g is not None:
                        for _ in g:
                            pass

                prev_gen = None
                jobs = [(1, hp) for hp in range(NPAIR)] + [(0, hp) for hp in range(NPAIR)]
                for qs, hp in jobs:
                    q0 = qs * FDT
                    jmax = NMSK * qs + NMSK - 1
                    if qs == 1:
                        emit_ut(hp)
                    if qs == 0 and hp in (1, 2, 3, 4, 5):
                        i = hp - 1
                        hi = NCT if hp == 5 else i + 1
                        emit_proj(1, range(i, hi))  # big slice's projection as filler
                    # scores: A.T[j, q] single K=64 matmuls, row-packed pairs;
                    # exp on the valid q-range only; mask-mult on the diag band
                    pb = p_pool.tile(
                        [P, 2, NJT, FDT], MM_DT, tag="pb", name="pb"
                    )
                    for jt in range(jmax + 1):
                        jrel = jt - NMSK * qs  # diag position (>=0 on diag tiles)
                        bs = max(jrel, 0) * P  # first valid q column
                        ap2 = a_ps.tile([P, 2, FDT], DT, tag="ap2", name="ap2")
                        for hh in range(2):
                            lo = hh * HD
                            nc.tensor.matmul(
                                ap2[:, hh, :],
                                uT_t[hp][jt // NMSK][
                                    lo : lo + HD, (jt % NMSK) * P : (jt % NMSK + 1) * P
                                ],
                                vT_sb[lo : lo + HD, hp, q0 : q0 + FDT],
                                start=True,
                                stop=True,
                            )
                        nc.scalar.activation(
                            pb[:, :, jt, bs:FDT],
                            ap2[:, :, bs:FDT],
                            mybir.ActivationFunctionType.Exp,
                            bias=bc_sb[:, jt : jt + 1],
                        )
                        if jrel >= 0:  # diagonal tile: mask the 128-wide band
                            for hh in range(2):
                                nc.vector.tensor_mul(
                                    pb[:, hh, jt, bs : bs + P],
                                    pb[:, hh, jt, bs : bs + P],
                                    mask_sb,
                                )
                        if prev_gen is not None:
                            for _ in range(4 if qs == 1 else 6):
                                if next(prev_gen, "end") == "end":
                                    prev_gen = None
                                    break
                    exhaust(prev_gen)
                    prev_gen = pv_gen(hp, qs, pb, jmax)
                exhaust(prev_gen)
                emit_proj(0)

    return nc

_NC = None


def build_nc():
    global _NC
    if _NC is None:
        nc = bacc.Bacc("TRN2", target_bir_lowering=False, debug=False)
        _emit(nc)
        nc.compile()
        _NC = nc
    return _NC


def make_mask128():
    import ml_dtypes

    m = (np.arange(P)[:, None] <= np.arange(P)[None, :]).astype(np.float32)
    return m.astype(ml_dtypes.bfloat16)


def host_inputs(x, Wv, bv, Wl, Wc, bc, Wp, bp):
    """Per-core input maps: layout/dtype prep (transposes + bf16 casts)."""
    import ml_dtypes

    bf16 = ml_dtypes.bfloat16
    x = np.ascontiguousarray(np.asarray(x, dtype=np.float32))
    shared = {
        "WvT": np.ascontiguousarray(np.asarray(Wv, np.float32).T.astype(bf16)),
        "WlT": np.ascontiguousarray(np.asarray(Wl, np.float32).T.astype(bf16)),
        "WcT": np.ascontiguousarray(
            (np.asarray(Wc, np.float32).T / np.sqrt(np.float32(HD))).astype(bf16)
        ),
        "WpT": np.ascontiguousarray(np.asarray(Wp, np.float32).T.astype(bf16)),
        "bv": np.ascontiguousarray(np.asarray(bv, np.float32)),
        "bc": np.ascontiguousarray(np.asarray(bc, np.float32)),
        "bp": np.ascontiguousarray(np.asarray(bp, np.float32)),
        "mask128": make_mask128(),
    }
    return [
        {"xT": np.ascontiguousarray(x[b].T.astype(bf16)), **shared} for b in range(B)
    ]


def kernel(x, Wv, bv, Wl, Wc, bc, Wp, bp, _trace=False):
    global LAST_EXEC_NS, LAST_RESULTS
    nc = build_nc()
    in_maps = host_inputs(x, Wv, bv, Wl, Wc, bc, Wp, bp)
    rb = run_bass_kernel_spmd(nc, in_maps, list(range(B)), trace=_trace)
    LAST_EXEC_NS = rb.exec_time_ns
    LAST_RESULTS = rb
    out = np.stack([rb.results[b]["outT"].T for b in range(B)], axis=0)
    return np.ascontiguousarray(out.astype(np.float32))


# revision 34
# speedup vs baseline: 1.4806x; 1.0276x over previous
"""Trainium2 Bass kernel for CausalSelfAttention with learned TxT score mixing.

Reference computation (per batch b):
    v = x @ Wv.T + bv ; q = k = v
    S = (v_h @ v_h.T) / sqrt(hd)            per head h   [T,T]
    A = S @ Wl.T @ Wc.T + bc                             [T,T]
    P = softmax(causal_mask(A))                          [T,T]
    y_h = P @ v_h ; out = concat(y) @ Wp.T + bp

Key algebra:
  * q == k == v makes S rank-64, so the TxT mixing collapses:
    A.T = Wc @ Wl @ (v_h v_h.T)/8 = ((Wc/8) @ (Wl @ v_h)) @ v_h.T = U_h @ v_h.T.
    We compute G_h = Wl @ v_h (per head, pair-packed) and U_h = (Wc/8) @ G_h
    directly -- no TxT @ TxT precompute at all.  U.T comes out of the second
    matmul in exactly the [d, j] layout the score matmuls need.
  * Scores stay in [key, query] layout; softmax uses unnormalized exp() and
    the normalizer Z[q] falls out of the PV matmul via a ones-column in the
    stationary operand (logits are O(1); masked entries are exactly zero).
  * Causal structure at 128 granularity: exp() runs only on the valid
    q-range of each key tile, the 0/1 mask multiply touches only the 128-wide
    diagonal band, and diagonal PV matmuls accumulate into the valid
    column sub-range only (the masked region never enters the sum).
  * Per-pair normalization: 1/Z rows are partition-broadcast with two K=1
    matmuls (col tile_position), no DRAM round-trips, so the tail is short.
  * Scheduling notes: dependencies are tile-granular, so DMA'd operands are
    split into per-slab tiles and U.T into per-(pair, j-half) tiles; U.T
    chains are emitted inside the job loop so the scalar engine's exp stream
    overlaps them; engine queues are in-order, so PSUM->SBUF drains are
    interleaved chain-by-chain with the matmul stream.

Sharding: data-parallel over batch (core b <- batch b). All weights
replicated; host passes transposed bf16 copies (layout/dtype prep only).
"""

import os
import sys

for _p in ("/opt/trn_rl_repo", "/root/.axon_site/_ro/trn_rl_repo"):
    if os.path.isdir(_p) and _p not in sys.path:
        sys.path.insert(0, _p)

import numpy as np

import concourse.bass as bass
import concourse.tile as tile
from concourse import bacc, mybir
from concourse.bass_utils import run_bass_kernel_spmd

B, T, C, NH = 8, 1024, 768, 12
HD = C // NH          # 64
P = 128               # SBUF partitions
NJT = T // P          # 8 j/k tiles of 128
NCT = C // P          # 6 c tiles of 128
NPAIR = NH // 2       # 6 head pairs (two heads share a 128-partition tile)
QG = 512              # q granularity for scores/exp/PV (= q slice width)
NQS = T // QG         # 2
FDT = QG
NMSK = QG // P        # 4 diagonal-tile positions per q slice
HC = HD + 1           # 65: head value columns + ones column for Z
DT = mybir.dt.float32
BF = mybir.dt.bfloat16
MM_DT = BF            # dtype of all matmul operands (accumulation stays fp32)

LAST_EXEC_NS = None
LAST_RESULTS = None


def _emit(nc):
    """Emit the whole per-core program inside a TileContext."""
    xT = nc.dram_tensor("xT", [C, T], MM_DT, kind="ExternalInput")
    WvT = nc.dram_tensor("WvT", [C, C], MM_DT, kind="ExternalInput")
    WlT = nc.dram_tensor("WlT", [T, T], MM_DT, kind="ExternalInput")
    WcT = nc.dram_tensor("WcT", [T, T], MM_DT, kind="ExternalInput")  # pre-scaled /8
    WpT = nc.dram_tensor("WpT", [C, C], MM_DT, kind="ExternalInput")
    bv = nc.dram_tensor("bv", [C], DT, kind="ExternalInput")
    bc = nc.dram_tensor("bc", [T], DT, kind="ExternalInput")
    bp = nc.dram_tensor("bp", [C], DT, kind="ExternalInput")
    mask128 = nc.dram_tensor("mask128", [P, P], MM_DT, kind="ExternalInput")
    outT = nc.dram_tensor("outT", [C, T], DT, kind="ExternalOutput")

    with tile.TileContext(nc) as tc:
        with tc.tile_pool(name="consts", bufs=1) as consts:
            vT_sb = consts.tile([P, NCT, T], MM_DT)          # v.T  [c, t]
            v_sb = consts.tile([P, NJT, NH * HC], MM_DT)     # v    [t, h*65+d], col 64 = ones
            # U.T pair-packed [hh*64+d, j], split per (pair, j-half) so each
            # score matmul depends on exactly one chain's copy
            uT_t = [
                [
                    consts.tile([P, FDT], MM_DT, name=f"uT_{hp}_{js}")
                    for js in range(NQS)
                ]
                for hp in range(NPAIR)
            ]
            yT_sb = consts.tile([P, NCT, T], MM_DT)          # normalized attn out, [c, t]
            wpT_t = consts.tile([P, NCT, C], MM_DT)          # Wp.T[c, c_out]
            wlT_t = consts.tile([P, NJT, T], MM_DT)          # Wl.T[t, m]
            wcT_t = consts.tile([P, NJT, T], MM_DT)          # Wc.T[m, j] (pre-scaled)
            g_sb = consts.tile([P, NJT, C], MM_DT)           # G[m, d] m-tile major
            mask_sb = consts.tile([P, P], MM_DT)             # lower-tri [jl, qq]: jl<=qq
            ones1_sb = consts.tile([1, HD], MM_DT)           # K=1 broadcast stationary
            bc_sb = consts.tile([P, NJT], DT)
            bv_sb = consts.tile([P, NCT], DT)
            bvbc_sb = consts.tile([P, C], DT)                # bv broadcast across partitions
            bp_sb = consts.tile([P, NCT], DT)

            # xT/wvT live in a scoped pool (read only until mid-kernel; later
            # pools may reuse the space -- that anti-dependency resolves long
            # before their first writes).  Per-slab tiles keep deps precise.
            ph1_cm = tc.tile_pool(name="ph1", bufs=1)
            ph1 = ph1_cm.__enter__()
            xT_t = [ph1.tile([P, T], MM_DT, name=f"xT_{ck}") for ck in range(NCT)]
            wvT_t = [ph1.tile([P, C], MM_DT, name=f"wvT_{ck}") for ck in range(NCT)]

            # Input DMAs fan out over three engine queues so the phase-1
            # operands (wvT + xT) land first and in parallel.
            for ck in range(NCT):
                nc.sync.dma_start(out=wvT_t[ck], in_=WvT[ck * P : (ck + 1) * P, :])
                nc.gpsimd.dma_start(out=xT_t[ck], in_=xT[ck * P : (ck + 1) * P, :])
            bv_ap = bv[:]
            nc.scalar.dma_start(
                out=bvbc_sb,
                in_=bass.AP(
                    tensor=bv_ap.tensor, offset=bv_ap.offset, ap=[[0, P]] + list(bv_ap.ap)
                ),
            )
            nc.scalar.dma_start(out=bv_sb, in_=bv[:].rearrange("(ct p) -> p ct", p=P))
            nc.scalar.dma_start(out=bc_sb, in_=bc[:].rearrange("(jt p) -> p jt", p=P))
            nc.scalar.dma_start(out=bp_sb, in_=bp[:].rearrange("(ct p) -> p ct", p=P))
            nc.scalar.dma_start(out=mask_sb, in_=mask128[:, :])
            for mt in range(NJT):
                nc.sync.dma_start(out=wlT_t[:, mt], in_=WlT[mt * P : (mt + 1) * P, :])
                nc.gpsimd.dma_start(out=wcT_t[:, mt], in_=WcT[mt * P : (mt + 1) * P, :])
            for ck in range(NCT):
                nc.sync.dma_start(out=wpT_t[:, ck], in_=WpT[ck * P : (ck + 1) * P, :])

            # ones columns of v_sb (column h*65+64 <- 1.0), for the Z row of PV
            for tt in range(NJT):
                nc.vector.memset(
                    v_sb[:, tt].rearrange("p (h e) -> p h e", e=HC)[:, :, HD : HD + 1],
                    1.0,
                )
            nc.vector.memset(ones1_sb, 1.0)

            # ---------------- phase 1: v (natural) and v.T projections ------------
            # chain-contiguous: each PSUM tile's 6-matmul chain completes before
            # the next begins, so the PSUM->SBUF bias-add drains interleave with
            # the matmul stream instead of bunching at phase end.
            with tc.tile_pool(name="ps1", bufs=8, space="PSUM") as ps1:
                # v.T[c, t] = sum_c' Wv[c, c'] x[t, c']  (+ bv[c] per-partition)
                for ts in range(2):
                    for ct in range(NCT):
                        pt = ps1.tile([P, FDT], DT, tag="pts1", name="pts1")
                        for ck in range(NCT):
                            nc.tensor.matmul(
                                pt,
                                wvT_t[ck][:, ct * P : (ct + 1) * P],
                                xT_t[ck][:, ts * FDT : (ts + 1) * FDT],
                                start=(ck == 0),
                                stop=(ck == NCT - 1),
                            )
                        nc.vector.tensor_scalar_add(
                            vT_sb[:, ct, ts * FDT : (ts + 1) * FDT],
                            pt,
                            bv_sb[:, ct : ct + 1],
                        )

                # v[t, c] = sum_c' x[t, c'] Wv[c, c']  (+ bv[c] broadcast)
                cslices = [(0, FDT), (FDT, C - FDT)]
                for tt in range(NJT):
                    for si, (c0, cw) in enumerate(cslices):
                        pt = ps1.tile([P, FDT], DT, tag="pts1", name="pts1")
                        for ck in range(NCT):
                            nc.tensor.matmul(
                                pt[:, :cw],
                                xT_t[ck][:, tt * P : (tt + 1) * P],
                                wvT_t[ck][:, c0 : c0 + cw],
                                start=(ck == 0),
                                stop=(ck == NCT - 1),
                            )
                        vdst = v_sb[:, tt].rearrange("p (h e) -> p h e", e=HC)
                        nh0, nh1 = c0 // HD, (c0 + cw) // HD
                        nc.vector.tensor_add(
                            vdst[:, nh0:nh1, 0:HD],
                            pt[:, :cw].rearrange("p (h e) -> p h e", e=HD),
                            bvbc_sb[:, c0 : c0 + cw].rearrange("p (h e) -> p h e", e=HD),
                        )
            ph1_cm.__exit__(None, None, None)

            # ---------------- phase 2a: G = Wl @ v ------------------------------
            # G[m, d] = sum_t Wl[m, t] v[t, d]       lhsT = Wl.T[t, m]  rhs = v[t, d]
            # (rhs walks v's head columns with a strided AP skipping the ones
            # columns, split 512 + 256 to fit PSUM banks)
            hslices = [(0, 8), (8, 4)]
            with tc.tile_pool(name="ps2", bufs=2, space="PSUM") as ps2:
                for mt in range(NJT):
                    gp = [
                        ps2.tile([P, nh * HD], DT, tag=f"gp{si}", name="gp")
                        for si, (_, nh) in enumerate(hslices)
                    ]
                    for kt in range(NJT):
                        vv = v_sb[:, kt].rearrange("p (h e) -> p h e", e=HC)
                        for si, (h0, nh) in enumerate(hslices):
                            nc.tensor.matmul(
                                gp[si],
                                wlT_t[:, kt, mt * P : (mt + 1) * P],
                                vv[:, h0 : h0 + nh, 0:HD],
                                start=(kt == 0),
                                stop=(kt == NJT - 1),
                            )
                    for si, (h0, nh) in enumerate(hslices):
                        nc.vector.tensor_copy(
                            g_sb[:, mt, h0 * HD : (h0 + nh) * HD], gp[si]
                        )

            # ---------------- phase 2b: U.T / scores / exp / mask / PV / norm ----
            # U.T[d, j] = sum_m G[m, d] WcT[m, j]    lhsT = G[m, d]   rhs = WcT[m, j]
            # U.T chains are emitted per-pair inside the job loop, so the exp
            # stream (ACT) of earlier pairs overlaps later pairs' U.T matmuls.
            with (
                tc.tile_pool(name="sm", bufs=2) as sm,
                tc.tile_pool(name="p_pool", bufs=3) as p_pool,
                tc.tile_pool(name="outp", bufs=4) as outp,
                tc.tile_pool(name="a_ps", bufs=2, space="PSUM") as a_ps,
                tc.tile_pool(name="y_ps", bufs=2, space="PSUM") as y_ps,
                tc.tile_pool(name="ps3", bufs=2, space="PSUM") as ps3,
            ):
                def emit_ut(hp):
                    for js in range(NQS):  # js=0 first: q-slice-0 tiles ready first
                        up = ps3.tile([P, FDT], DT, tag="pp", name="up")
                        for mt in range(NJT):
                            nc.tensor.matmul(
                                up,
                                g_sb[:, mt, hp * P : (hp + 1) * P],
                                wcT_t[:, mt, js * FDT : (js + 1) * FDT],
                                start=(mt == 0),
                                stop=(mt == NJT - 1),
                            )
                        nc.vector.tensor_copy(uT_t[hp][js], up)

                def emit_proj(ts, cts=range(NCT)):
                    # outT[c_out, t] = Wp @ yT (+bp), ct-outer accumulation chains
                    for ct in cts:
                        pp = ps3.tile([P, FDT], DT, tag="pp")
                        for ck in range(NCT):
                            nc.tensor.matmul(
                                pp,
                                wpT_t[:, ck, ct * P : (ct + 1) * P],
                                yT_sb[:, ck, ts * FDT : (ts + 1) * FDT],
                                start=(ck == 0),
                                stop=(ck == NCT - 1),
                            )
                        ot = outp.tile([P, FDT], DT, tag="ot")
                        # bias-add on ACT: projections run in windows where the
                        # exp stream is idle, and the vector queue is the
                        # binding engine there
                        nc.scalar.activation(
                            ot,
                            pp,
                            mybir.ActivationFunctionType.Identity,
                            bias=bp_sb[:, ct : ct + 1],
                        )
                        nc.gpsimd.dma_start(
                            out=outT[ct * P : (ct + 1) * P, ts * FDT : (ts + 1) * FDT],
                            in_=ot,
                        )

                def pv_gen(hp, qs, pb, jmax):
                    """Generator emitting the PV chains + per-pair normalization;
                    driven interleaved with the NEXT pair's score stream so the
                    PE keeps busy while ACT runs this pair's exps."""
                    q0 = qs * FDT
                    zrec = [
                        sm.tile([1, FDT], MM_DT, tag=f"zrec{hh}", name="zrec")
                        for hh in range(2)
                    ]
                    yu = sm.tile([P, FDT], MM_DT, tag="yu", name="yu", bufs=2)
                    for hh in range(2):
                        h = hp * 2 + hh
                        yp = y_ps.tile([HC, QG], DT, tag="yp", name="yp")
                        for kt in range(jmax + 1):
                            # diagonal k-tiles contribute only to q >= bs; the
                            # masked-out columns never enter the accumulation
                            bs = max(kt - NMSK * qs, 0) * P
                            nc.tensor.matmul(
                                yp[:, bs:FDT],
                                v_sb[:, kt, h * HC : (h + 1) * HC],
                                pb[:, hh, kt, bs:FDT],
                                start=(kt == 0),
                                stop=(kt == jmax),
                            )
                            yield
                        # 1/Z for this head (bf16 row, feeds the broadcast mm);
                        # approx-fast recip off a PSUM-staged copy -- a plain
                        # single-partition reciprocal costs ~3.3us on DVE.
                        # In the small-slice phase ACT has slack and the vector
                        # queue binds, so the copy/cast legs run on ACT there.
                        zs = sm.tile([1, FDT], DT, tag="zs", name="zs")
                        zf = sm.tile([1, FDT], DT, tag="zf", name="zf")
                        on_act = qs == 0 or hp == NPAIR - 1
                        if on_act:
                            nc.scalar.activation(
                                zs,
                                yp[HD : HD + 1, :],
                                mybir.ActivationFunctionType.Identity,
                            )
                        else:
                            nc.vector.tensor_copy(zs, yp[HD : HD + 1, :])
                        nc.vector.reciprocal_approx_fast(zf, zs)
                        if on_act:
                            nc.scalar.activation(
                                zrec[hh],
                                zf,
                                mybir.ActivationFunctionType.Identity,
                            )
                        else:
                            nc.vector.tensor_copy(zrec[hh], zf)
                        if hh == 0:
                            nc.vector.tensor_copy(yu[0:HD, :], yp[0:HD, :])
                        else:
                            stg = sm.tile([HD, QG], MM_DT, tag="stg", name="stg")
                            nc.vector.tensor_copy(stg, yp[0:HD, :])
                            nc.sync.dma_start(out=yu[HD:P, :], in_=stg)
                        yield
                    # partition-broadcast of 1/Z via two K=1 matmuls, then normalize
                    rbb = ps3.tile([P, FDT], DT, tag="pp", name="rbb")
                    nc.tensor.matmul(
                        rbb[0:HD, :], ones1_sb, zrec[0], start=True, stop=True
                    )
                    nc.tensor.matmul(
                        rbb[HD:P, :],
                        ones1_sb,
                        zrec[1],
                        start=True,
                        stop=True,
                        tile_position=(0, HD),
                    )
                    nc.vector.tensor_mul(yT_sb[:, hp, q0 : q0 + FDT], rbb, yu)
                    yield

                def exhaust(g):
                    if g is not None:
                        for _ in g:
                            pass

                prev_gen = None
                jobs = [(1, hp) for hp in range(NPAIR)] + [(0, hp) for hp in range(NPAIR)]
                for qs, hp in jobs:
                    q0 = qs * FDT
                    jmax = NMSK * qs + NMSK - 1
                    if qs == 1:
                        emit_ut(hp)
                    if qs == 0 and hp in (1, 2, 3, 4, 5):
                        i = hp - 1
                        hi = NCT if hp == 5 else i + 1
                        emit_proj(1, range(i, hi))  # big slice's projection as filler
                    # scores: A.T[j, q] single K=64 matmuls, row-packed pairs;
                    # exp on the valid q-range only; mask-mult on the diag band
                    pb = p_pool.tile(
                        [P, 2, NJT, FDT], MM_DT, tag="pb", name="pb"
                    )
                    for jt in range(jmax + 1):
                        jrel = jt - NMSK * qs  # diag position (>=0 on diag tiles)
                        bs = max(jrel, 0) * P  # first valid q column
                        ap2 = a_ps.tile([P, 2, FDT], DT, tag="ap2", name="ap2")
                        for hh in range(2):
                            lo = hh * HD
                            nc.tensor.matmul(
                                ap2[:, hh, :],
                                uT_t[hp][jt // NMSK][
                                    lo : lo + HD, (jt % NMSK) * P : (jt % NMSK + 1) * P
                                ],
                                vT_sb[lo : lo + HD, hp, q0 : q0 + FDT],
                                start=True,
                                stop=True,
                            )
                        nc.scalar.activation(
                            pb[:, :, jt, bs:FDT],
                            ap2[:, :, bs:FDT],
                            mybir.ActivationFunctionType.Exp,
                            bias=bc_sb[:, jt : jt + 1],
                        )
                        if jrel >= 0:  # diagonal tile: mask the 128-wide band
                            for hh in range(2):
                                nc.vector.tensor_mul(
                                    pb[:, hh, jt, bs : bs + P],
                                    pb[:, hh, jt, bs : bs + P],
                                    mask_sb,
                                )
                        if prev_gen is not None:
                            for _ in range(4 if qs == 1 else 6):
                                if next(prev_gen, "end") == "end":
                                    prev_gen = None
                                    break
                    exhaust(prev_gen)
                    prev_gen = pv_gen(hp, qs, pb, jmax)
                exhaust(prev_gen)
                emit_proj(0)

    return nc

_NC = None


def build_nc():
    global _NC
    if _NC is None:
        nc = bacc.Bacc("TRN2", target_bir_lowering=False, debug=False)
        _emit(nc)
        nc.compile()
        _NC = nc
    return _NC


def make_mask128():
    import ml_dtypes

    m = (np.arange(P)[:, None] <= np.arange(P)[None, :]).astype(np.float32)
    return m.astype(ml_dtypes.bfloat16)


def host_inputs(x, Wv, bv, Wl, Wc, bc, Wp, bp):
    """Per-core input maps: layout/dtype prep (transposes + bf16 casts)."""
    import ml_dtypes

    bf16 = ml_dtypes.bfloat16
    x = np.ascontiguousarray(np.asarray(x, dtype=np.float32))
    shared = {
        "WvT": np.ascontiguousarray(np.asarray(Wv, np.float32).T.astype(bf16)),
        "WlT": np.ascontiguousarray(np.asarray(Wl, np.float32).T.astype(bf16)),
        "WcT": np.ascontiguousarray(
            (np.asarray(Wc, np.float32).T / np.sqrt(np.float32(HD))).astype(bf16)
        ),
        "WpT": np.ascontiguousarray(np.asarray(Wp, np.float32).T.astype(bf16)),
        "bv": np.ascontiguousarray(np.asarray(bv, np.float32)),
        "bc": np.ascontiguousarray(np.asarray(bc, np.float32)),
        "bp": np.ascontiguousarray(np.asarray(bp, np.float32)),
        "mask128": make_mask128(),
    }
    return [
        {"xT": np.ascontiguousarray(x[b].T.astype(bf16)), **shared} for b in range(B)
    ]


def kernel(x, Wv, bv, Wl, Wc, bc, Wp, bp, _trace=False):
    global LAST_EXEC_NS, LAST_RESULTS
    nc = build_nc()
    in_maps = host_inputs(x, Wv, bv, Wl, Wc, bc, Wp, bp)
    rb = run_bass_kernel_spmd(nc, in_maps, list(range(B)), trace=_trace)
    LAST_EXEC_NS = rb.exec_time_ns
    LAST_RESULTS = rb
    out = np.stack([rb.results[b]["outT"].T for b in range(B)], axis=0)
    return np.ascontiguousarray(out.astype(np.float32))
